# revision 13
# baseline (speedup 1.0000x reference)
"""Trainium2 Bass kernel for nn_CASAtt_MultiHead_v1 (CAS attention block).

Reference computation (per sample):
    qkv = 1x1 conv (qkv_w) -> q, k, v                        [512, 56, 56] each
    q <- SE(dwconv3x3(q, sq_w, sq_b))   (per-head squeeze-excite)
    k <- SE(dwconv3x3(k, sk_w, sk_b))
    out = proj(dwconv3x3(q + k, dwc_w, dwc_b) * v) + proj_b + x

Distribution: data-parallel over batch, 2 samples per NeuronCore x 8 cores.

Layout: channels on partitions, 4 chunks of 128 (chunk == SE head).
GEMMs in bf16 on the PE (fp32 PSUM).  Depthwise 3x3 convs run either as
9 diagonal-matrix matmuls on the PE, or as scalar-MAC tap chains on the
vector engines over contiguous padded-flat slices (WP=59 so 5 of 9 tap
offsets are 4B-aligned and the DVE 2x bf16 mode engages; odd-parity taps
either run 1x STT on DVE or are offloaded as scaled copies to the
scalar/pool engines and accumulated with aligned 2x tensor_tensor adds).
Engine per (conv, head) is cfg-tunable to balance PE vs DVE vs ACT vs
POOL.  The residual (+x) is accumulated into the proj PSUM group via an
identity-matrix matmul, so the final drain is a single ACT activation.
m = s_q*dwq + s_k*dwk is built in-place in dwk; a single third conv runs
on m.  Mixed-dtype tensor-tensor DVE ops (psum f32 + bf16 operand)
produce NaN on hardware -- every tensor-tensor-class op keeps both
tensor operands the same dtype.
"""

import numpy as np

DIM = 512
NH = 4
HD = 128
HD4 = 32
B, H_FULL, W = 16, 56, 56
N_CORES = 8

TAPS = [(dy, dx) for dy in (-1, 0, 1) for dx in (-1, 0, 1)]


def default_cfg():
    # conv_assign: engine per (conv_id, head).  conv_id: 0=q, 1=k, 2=m.
    #   'pe'  diag matmuls on TensorE
    #   'dve' pure DVE tap chain (odd taps 1x STT)
    #   'dva' odd taps as ACT scaled copies + DVE 2x adds
    #   'dvp' odd taps as POOL scaled copies + DVE 2x adds
    assign = {
        (0, 0): 'dva', (0, 1): 'dvp', (0, 2): 'dva', (0, 3): 'dvp',
        (1, 0): 'dva', (1, 1): 'dvp', (1, 2): 'dva', (1, 3): 'pe',
        (2, 0): 'dva', (2, 1): 'dvp', (2, 2): 'dva', (2, 3): 'pe',
    }
    return dict(
        b_local=B // N_CORES,
        H=H_FULL,
        rows_per_tile=8,
        conv_assign=assign,
    )


def build_nc(cfg):
    """Build + compile the Bacc program for one core (SPMD across 8)."""
    import concourse.bass as bass
    import concourse.mybir as mybir
    import concourse.tile as tile
    from concourse import bacc
    from contextlib import ExitStack

    f32 = mybir.dt.float32
    bf16 = mybir.dt.bfloat16
    cdt = bf16
    gdt = bf16

    BL = cfg['b_local']
    H = cfg['H']
    TH = cfg['rows_per_tile']
    NT = H // TH
    assert NT * TH == H
    TN = TH * W
    HP, WP = H + 2, W + 3
    PADN = HP * WP
    TPAD = TH * WP
    AF = mybir.ActivationFunctionType
    AL = mybir.AluOpType
    assign = cfg['conv_assign']
    # taps ordered: even-parity (2x-capable) first, odds last; final tap
    # is strided (1x regardless) so it takes an odd slot
    evens = [j for j, (dy, dx) in enumerate(TAPS) if (dy * WP + dx) % 2 == 0]
    odds = [j for j, (dy, dx) in enumerate(TAPS) if (dy * WP + dx) % 2]

    nc = bacc.Bacc("TRN2", target_bir_lowering=False, debug=False,
                   enable_asserts=False, num_devices=N_CORES)

    # ---------------- DRAM I/O ----------------
    x_d = nc.dram_tensor("x", [BL, DIM, H, W], gdt, kind="ExternalInput").ap()
    out_d = nc.dram_tensor("out", [BL, DIM, H, W], f32, kind="ExternalOutput").ap()
    wq_d = nc.dram_tensor("wq_t", [DIM, DIM], gdt, kind="ExternalInput").ap()
    wk_d = nc.dram_tensor("wk_t", [DIM, DIM], gdt, kind="ExternalInput").ap()
    wv_d = nc.dram_tensor("wv_t", [DIM, DIM], gdt, kind="ExternalInput").ap()
    wp_d = nc.dram_tensor("proj_t", [DIM, DIM], gdt, kind="ExternalInput").ap()
    ident_d = nc.dram_tensor("ident", [HD, HD], gdt, kind="ExternalInput").ap()
    dg_d = [nc.dram_tensor(n, [NH, 9, HD, HD], cdt, kind="ExternalInput").ap()
            for n in ("diag1q", "diag1k", "diag2")]
    wv1_d = [nc.dram_tensor(n, [NH, HD, 9], f32, kind="ExternalInput").ap()
             for n in ("wvec1q", "wvec1k", "wvec2")]
    b1_d = [nc.dram_tensor(n, [DIM, 1], f32, kind="ExternalInput").ap()
            for n in ("sq_b", "sk_b", "dwc_b")]
    projb_d = nc.dram_tensor("proj_b", [DIM, 1], f32, kind="ExternalInput").ap()
    sew1_d = [nc.dram_tensor(n, [NH, HD, HD4], f32, kind="ExternalInput").ap()
              for n in ("se_w1q", "se_w1k")]
    seb1_d = [nc.dram_tensor(n, [NH, HD4, 1], f32, kind="ExternalInput").ap()
              for n in ("se_b1q", "se_b1k")]
    sew2_d = [nc.dram_tensor(n, [NH, HD4, HD], f32, kind="ExternalInput").ap()
              for n in ("se_w2q", "se_w2k")]
    seb2_d = [nc.dram_tensor(n, [NH, HD, 1], f32, kind="ExternalInput").ap()
              for n in ("se_b2q", "se_b2k")]

    with tile.TileContext(nc) as tc, ExitStack() as ctx:
        const = ctx.enter_context(tc.tile_pool(name="const", bufs=1))
        big = ctx.enter_context(tc.tile_pool(name="big", bufs=1))
        wpool = ctx.enter_context(tc.tile_pool(name="wpool", bufs=1))
        xpool = ctx.enter_context(tc.tile_pool(name="xpool", bufs=2))
        vpool = ctx.enter_context(tc.tile_pool(name="vpool", bufs=2))
        o2pool = ctx.enter_context(tc.tile_pool(name="o2pool", bufs=2))
        otpool = ctx.enter_context(tc.tile_pool(name="otpool", bufs=2))
        statpool = ctx.enter_context(tc.tile_pool(name="statpool", bufs=16))
        mmpool = ctx.enter_context(tc.tile_pool(name="mmpool", bufs=5, space="PSUM"))
        sepool = ctx.enter_context(tc.tile_pool(name="sepool", bufs=2, space="PSUM"))

        # ---------- persistent SBUF ----------
        # padded conv-domain buffers; 2-elem slop so padded-space tap reads
        # (offsets -WP-1 .. +WP+1) stay in bounds
        qpad = [big.tile([HD, PADN + 2], cdt, name=f"qpad{c}") for c in range(NH)]
        kpad = [big.tile([HD, PADN + 2], cdt, name=f"kpad{c}") for c in range(NH)]
        dwq = [big.tile([HD, PADN + 2], cdt, name=f"dwq{c}") for c in range(NH)]
        dwk = [big.tile([HD, PADN + 2], cdt, name=f"dwk{c}") for c in range(NH)]

        def pad3(t):
            return t[:, 1:1 + PADN].rearrange("p (h w) -> p h w", w=WP)

        qpad3, kpad3 = [pad3(t) for t in qpad], [pad3(t) for t in kpad]
        dwq3, dwk3 = [pad3(t) for t in dwq], [pad3(t) for t in dwk]

        # flat conv2 outputs for off-PE conv2 heads
        c2flat = {}
        for oc in range(NH):
            if assign[(2, oc)] != 'pe':
                c2flat[oc] = big.tile([HD, H * W], cdt, name=f"c2f{oc}")

        # accumulator + odd-tap scratch for off-PE convs (pad cells may
        # hold stale garbage between uses; only interior is consumed)
        acc_g = big.tile([HD, PADN + 2], cdt, name="accg")
        tmp_g = [big.tile([HD, PADN + 2], cdt, name=f"tmpg{i}") for i in range(2)]

        for tt in qpad + kpad + dwq + dwk + [acc_g] + tmp_g:
            nc.vector.memset(tt, 0.0)

        # small constants
        bias1 = [[const.tile([HD, 1], f32, name=f"b1_{ci}_{c}") for c in range(NH)]
                 for ci in range(3)]
        projb = [const.tile([HD, 1], f32, name=f"projb{c}") for c in range(NH)]
        for c in range(NH):
            sl = slice(c * HD, (c + 1) * HD)
            for ci in range(3):
                nc.sync.dma_start(bias1[ci][c], b1_d[ci][sl])
            nc.sync.dma_start(projb[c], projb_d[sl])
        sew1 = [[const.tile([HD, HD4], f32, name=f"sew1_{br}_{c}") for c in range(NH)]
                for br in range(2)]
        seb1 = [[const.tile([HD4, 1], f32, name=f"seb1_{br}_{c}") for c in range(NH)]
                for br in range(2)]
        sew2 = [[const.tile([HD4, HD], f32, name=f"sew2_{br}_{c}") for c in range(NH)]
                for br in range(2)]
        seb2 = [[const.tile([HD, 1], f32, name=f"seb2_{br}_{c}") for c in range(NH)]
                for br in range(2)]
        wvec = [[const.tile([HD, 9], f32, name=f"wvec_{ci}_{c}") for c in range(NH)]
                for ci in range(3)]
        for br in range(2):
            for c in range(NH):
                nc.sync.dma_start(sew1[br][c], sew1_d[br][c])
                nc.sync.dma_start(seb1[br][c], seb1_d[br][c])
                nc.sync.dma_start(sew2[br][c], sew2_d[br][c])
                nc.sync.dma_start(seb2[br][c], seb2_d[br][c])
        for ci in range(3):
            for c in range(NH):
                nc.sync.dma_start(wvec[ci][c], wv1_d[ci][c])

        # persistent GEMM weights (loaded once, reused across samples)
        ident = const.tile([HD, HD], gdt, name="ident")
        nc.sync.dma_start(ident, ident_d)

        def load_w(w_d, nm):
            w_sb = []
            for kc in range(NH):
                row = []
                for oc in range(NH):
                    wt = wpool.tile([HD, HD], gdt, name=f"{nm}{kc}_{oc}")
                    nc.sync.dma_start(wt, w_d[kc * HD:(kc + 1) * HD,
                                              oc * HD:(oc + 1) * HD])
                    row.append(wt)
                w_sb.append(row)
            return w_sb

        wq_sb = load_w(wq_d, "wq")
        wk_sb = load_w(wk_d, "wk")
        wv_sb = load_w(wv_d, "wv")
        wp_sb = load_w(wp_d, "wp")

        # persistent diag weights for PE-assigned conv heads
        dg_sb = {}
        for ci in range(3):
            for oc in range(NH):
                if assign[(ci, oc)] == 'pe':
                    dg_sb[(ci, oc)] = [
                        const.tile([HD, HD], cdt, name=f"dg{ci}_{oc}_{j}")
                        for j in range(9)]
                    for j in range(9):
                        nc.sync.dma_start(dg_sb[(ci, oc)][j], dg_d[ci][oc, j])

        def taps_flat_tile(tbuf, r0):
            """9 contiguous slices (full padded rows) for padded-space conv
            over output padded rows r0+1..r0+TH (tile granularity, PE)."""
            base = 1 + (r0 + 1) * WP
            return [tbuf[:, base + dy * WP + dx: base + dy * WP + dx + TPAD]
                    for (dy, dx) in TAPS]

        def tap_bounds(j):
            """Whole-chunk padded-flat bounds for tap j: covers padded rows
            1..H, start/count adjusted to even element parity."""
            dy, dx = TAPS[j]
            delta = dy * WP + dx
            lo, cnt = WP, H * WP
            if (1 + lo + delta) % 2:
                lo, cnt = lo + 1, cnt - 1
            if cnt % 2:
                cnt -= 1
            return lo, cnt, delta

        # canonical aligned superset of the interior for 2x TT accumulate
        # (element start 1+CLO = 60 is 4B-aligned; cells outside the
        # interior get garbage, which is never read)
        CLO, CCNT = WP, H * WP

        def conv_head(ci, oc, srcb, src3, dst3_or_flat, b, stats, eng,
                     want_stats):
            """One depthwise-conv head.  dst3_or_flat: 3-d padded view
            (conv1) or flat [HD, H*W] tile (conv2)."""
            bias = bias1[ci][oc]
            wv_ = wvec[ci][oc]
            sfx = f"{b}_{ci}_{oc}"
            if eng == 'pe':
                return  # handled separately (emission order differs)
            # evens: first with bias via TS, rest in-place STT (2x)
            j0 = evens[0]
            lo, cnt, delta = tap_bounds(j0)
            nc.vector.tensor_scalar(
                acc_g[:, 1 + lo:1 + lo + cnt],
                srcb[:, 1 + lo + delta:1 + lo + delta + cnt],
                wv_[:, j0:j0 + 1], bias, AL.mult, AL.add)
            for jj in evens[1:]:
                lo, cnt, delta = tap_bounds(jj)
                nc.vector.scalar_tensor_tensor(
                    acc_g[:, 1 + lo:1 + lo + cnt],
                    srcb[:, 1 + lo + delta:1 + lo + delta + cnt],
                    wv_[:, jj:jj + 1],
                    acc_g[:, 1 + lo:1 + lo + cnt],
                    AL.mult, AL.add)
            # odd taps (all but the final one): first len(tmp_g) go as
            # off-DVE scaled copies + aligned 2x TT adds ('dva'/'dvp'),
            # the rest as in-place 1x STT on DVE
            n_off = len(tmp_g) if eng in ('dva', 'dvp') else 0
            for i, jj in enumerate(odds[:-1][:n_off]):
                lo, cnt, delta = tap_bounds(jj)
                t_sl = tmp_g[i][:, 1 + lo:1 + lo + cnt]
                src_sl = srcb[:, 1 + lo + delta:1 + lo + delta + cnt]
                if eng == 'dva':
                    nc.scalar.activation(t_sl, src_sl, AF.Copy,
                                         scale=wv_[:, jj:jj + 1])
                else:
                    nc.gpsimd.tensor_scalar(t_sl, src_sl,
                                            wv_[:, jj:jj + 1], None,
                                            AL.mult)
            for i, jj in enumerate(odds[:-1][:n_off]):
                nc.vector.tensor_tensor(
                    acc_g[:, 1 + CLO:1 + CLO + CCNT],
                    acc_g[:, 1 + CLO:1 + CLO + CCNT],
                    tmp_g[i][:, 1 + CLO:1 + CLO + CCNT],
                    AL.add)
            for jj in odds[:-1][n_off:]:
                lo, cnt, delta = tap_bounds(jj)
                nc.vector.scalar_tensor_tensor(
                    acc_g[:, 1 + lo:1 + lo + cnt],
                    srcb[:, 1 + lo + delta:1 + lo + delta + cnt],
                    wv_[:, jj:jj + 1],
                    acc_g[:, 1 + lo:1 + lo + cnt],
                    AL.mult, AL.add)
            # final odd tap: strided interior finalize
            j8 = odds[-1]
            dy, dx = TAPS[j8]
            acc3 = pad3(acc_g)
            kw = dict(accum_out=stats[:, 0:1]) if want_stats else {}
            if ci < 2:
                dst = dst3_or_flat[:, 1:1 + H, 1:1 + W]
            else:
                dst = dst3_or_flat.rearrange("p (h w) -> p h w", w=W)
            nc.vector.scalar_tensor_tensor(
                dst,
                src3[:, 1 + dy:1 + dy + H, 1 + dx:1 + dx + W],
                wv_[:, j8:j8 + 1],
                acc3[:, 1:1 + H, 1:1 + W],
                AL.mult, AL.add, **kw)

        def pe_conv_head(ci, oc, srcb, dst3, b, stats):
            """PE diag-matmul conv head (conv1 only: writes padded dst3
            with bias + accum pooling)."""
            dgs = dg_sb[(ci, oc)]
            for t in range(NT):
                r0 = t * TH
                ps = mmpool.tile([HD, TPAD], f32, tag="mm",
                                 name=f"c1{b}_{ci}_{t}_{oc}")
                for j, v in enumerate(taps_flat_tile(srcb, r0)):
                    nc.tensor.matmul(ps, dgs[j], v, start=(j == 0),
                                     stop=(j == 8))
                nc.scalar.activation(
                    dst3[:, 1 + r0:1 + r0 + TH, 1:1 + W],
                    ps.rearrange("p (h w) -> p h w", w=WP)[:, :, 1:1 + W],
                    AF.Identity, bias=bias1[ci][oc],
                    accum_out=stats[:, t:t + 1])

        def emit_se(b, br, oc, stats, pooled_w, s_scale):
            """SE chain for one head: pooled stats -> sigmoid scale."""
            pooled = const.tile([HD, 1], f32, tag="pooled", bufs=4,
                                name=f"pool{b}_{br}_{oc}")
            nc.vector.tensor_reduce(pooled, stats[:, 0:pooled_w],
                                    mybir.AxisListType.X, AL.add)
            ps1 = sepool.tile([HD4, 1], f32, tag="se", name=f"se1_{b}_{br}_{oc}")
            nc.tensor.matmul(ps1, sew1[br][oc], pooled, start=True, stop=True)
            hvec = const.tile([HD4, 1], f32, tag="hvec", bufs=4,
                              name=f"h{b}_{br}_{oc}")
            nc.scalar.activation(hvec, ps1, AF.Relu, bias=seb1[br][oc])
            ps2 = sepool.tile([HD, 1], f32, tag="se", name=f"se2_{b}_{br}_{oc}")
            nc.tensor.matmul(ps2, sew2[br][oc], hvec, start=True, stop=True)
            s_sb = const.tile([HD, 1], f32, tag="s_scale", bufs=16,
                              name=f"s{b}_{br}_{oc}")
            nc.scalar.activation(s_sb, ps2, AF.Sigmoid, bias=seb2[br][oc])
            s_scale[b][br][oc] = s_sb

        s_scale = [[[None] * NH for _ in range(2)] for _ in range(BL)]
        stats_t = {}

        def phaseA(b):
            """q,k GEMMs for sample b -> qpad/kpad."""
            for br in range(2):
                w_sb = wq_sb if br == 0 else wk_sb
                p3 = qpad3 if br == 0 else kpad3
                for t in range(NT):
                    r0 = t * TH
                    xt = []
                    for kc in range(NH):
                        xx = xpool.tile([HD, TN], gdt, tag=f"x{kc}",
                                        name=f"xa{kc}_b{b}_{br}_{t}")
                        nc.sync.dma_start(
                            xx.rearrange("p (h w) -> p h w", w=W),
                            x_d[b, kc * HD:(kc + 1) * HD, r0:r0 + TH, :])
                        xt.append(xx)
                    for oc in range(NH):
                        ps = mmpool.tile([HD, TN], f32, tag="mm",
                                         name=f"g{b}_{br}_{t}_{oc}")
                        for kc in range(NH):
                            nc.tensor.matmul(ps, w_sb[kc][oc], xt[kc],
                                             start=(kc == 0),
                                             stop=(kc == NH - 1))
                        nc.scalar.copy(
                            p3[oc][:, 1 + r0:1 + r0 + TH, 1:1 + W],
                            ps.rearrange("p (h w) -> p h w", w=W))

        def phaseB_offpe(b):
            """Off-PE conv1 heads (SE chains emitted later: their PE
            matmuls must not block the next sample's GEMMs in the PE
            queue)."""
            for br in range(2):
                srcb = qpad if br == 0 else kpad
                src3 = qpad3 if br == 0 else kpad3
                dst3 = dwq3 if br == 0 else dwk3
                for oc in range(NH):
                    eng = assign[(br, oc)]
                    if eng == 'pe':
                        continue
                    stats = statpool.tile([HD, NT], f32, tag="stats",
                                          name=f"st{b}_{br}_{oc}")
                    conv_head(br, oc, srcb[oc], src3[oc], dst3[oc], b,
                              stats, eng, True)
                    stats_t[(b, br, oc)] = stats

        def phaseB_offpe_se(b):
            for br in range(2):
                for oc in range(NH):
                    if assign[(br, oc)] == 'pe':
                        continue
                    emit_se(b, br, oc, stats_t[(b, br, oc)], 1, s_scale)

        def phaseB_pe(b):
            """PE conv1 heads + their SE chains."""
            for br in range(2):
                srcb = qpad if br == 0 else kpad
                dst3 = dwq3 if br == 0 else dwk3
                for oc in range(NH):
                    if assign[(br, oc)] != 'pe':
                        continue
                    stats = statpool.tile([HD, NT], f32, tag="stats",
                                          name=f"stp{b}_{br}_{oc}")
                    pe_conv_head(br, oc, srcb[oc], dst3[oc], b, stats)
                    emit_se(b, br, oc, stats, NT, s_scale)

        def phase15(b):
            """m = s_q*dwq + s_k*dwk, in place in dwk."""
            for oc in range(NH):
                nc.vector.tensor_scalar(dwk[oc], dwk[oc], s_scale[b][1][oc],
                                        None, AL.mult)
                nc.vector.scalar_tensor_tensor(dwk[oc], dwq[oc],
                                               s_scale[b][0][oc], dwk[oc],
                                               AL.mult, AL.add)

        def phaseC_offpe(b):
            """Off-PE conv2 heads -> c2flat."""
            for oc in range(NH):
                eng = assign[(2, oc)]
                if eng == 'pe':
                    continue
                conv_head(2, oc, dwk[oc], dwk3[oc], c2flat[oc], b,
                          None, eng, False)

        def phaseD(b):
            """Per row-tile: v GEMM, o2 = c2*v, proj GEMM + residual."""
            for t in range(NT):
                r0 = t * TH
                xt = []
                for kc in range(NH):
                    xx = xpool.tile([HD, TN], gdt, tag=f"x{kc}",
                                    name=f"xd{kc}_b{b}_{t}")
                    nc.sync.dma_start(
                        xx.rearrange("p (h w) -> p h w", w=W),
                        x_d[b, kc * HD:(kc + 1) * HD, r0:r0 + TH, :])
                    xt.append(xx)
                o2 = []
                for oc in range(NH):
                    ps = mmpool.tile([HD, TN], f32, tag="mm",
                                     name=f"v{b}_{t}_{oc}")
                    for kc in range(NH):
                        nc.tensor.matmul(ps, wv_sb[kc][oc], xt[kc],
                                         start=(kc == 0), stop=(kc == NH - 1))
                    vv = vpool.tile([HD, TN], gdt, tag=f"vt{oc}",
                                    name=f"vt{oc}_b{b}_{t}")
                    nc.scalar.copy(vv, ps)
                    if assign[(2, oc)] == 'pe':
                        # conv2 on PE for this head, this tile
                        ps2 = mmpool.tile([HD, TPAD], f32, tag="mm",
                                          name=f"c2{b}_{t}_{oc}")
                        for j, v in enumerate(taps_flat_tile(dwk[oc], r0)):
                            nc.tensor.matmul(ps2, dg_sb[(2, oc)][j], v,
                                             start=(j == 0), stop=(j == 8))
                        c2t = o2pool.tile([HD, TN], gdt, tag=f"c2t{oc}",
                                          name=f"c2t{oc}_b{b}_{t}")
                        nc.scalar.activation(
                            c2t.rearrange("p (h w) -> p h w", w=W),
                            ps2.rearrange("p (h w) -> p h w",
                                          w=WP)[:, :, 1:1 + W],
                            AF.Identity, bias=bias1[2][oc])
                        c2_sl = c2t
                    else:
                        c2_sl = c2flat[oc][:, r0 * W:r0 * W + TN]
                    oo = o2pool.tile([HD, TN], gdt, tag=f"o2_{oc}",
                                     name=f"o2_{oc}_b{b}_{t}")
                    nc.vector.tensor_mul(oo, c2_sl, vv)
                    o2.append(oo)
                for oc in range(NH):
                    ps = mmpool.tile([HD, TN], f32, tag="mm",
                                     name=f"p{b}_{t}_{oc}")
                    for kc in range(NH):
                        nc.tensor.matmul(ps, wp_sb[kc][oc], o2[kc],
                                         start=(kc == 0), stop=False)
                    # residual: accumulate x via identity matmul
                    nc.tensor.matmul(ps, ident, xt[oc], start=False,
                                     stop=True)
                    ot = otpool.tile([HD, TN], f32, tag="ot", bufs=3,
                                     name=f"ot{oc}_b{b}_{t}")
                    nc.scalar.activation(ot, ps, AF.Identity, bias=projb[oc])
                    nc.sync.dma_start(
                        out_d[b, oc * HD:(oc + 1) * HD, r0:r0 + TH, :],
                        ot.rearrange("p (h w) -> p h w", w=W))

        # ---- emission schedule: overlap sample b's vector-engine conv
        # phase with sample b+1's PE GEMM phase ----
        phaseA(0)
        for b in range(BL):
            phaseB_offpe(b)       # DVE/ACT/POOL conv1
            phaseB_pe(b)          # PE conv1 share + SE
            if b + 1 < BL:
                phaseA(b + 1)     # next sample's GEMMs fill the PE
            phaseB_offpe_se(b)
            phase15(b)
            phaseC_offpe(b)
            phaseD(b)

    nc.compile()
    return nc


# ---------------------------------------------------------------------------
# host-side weight prep
# ---------------------------------------------------------------------------

def prep_weights(inputs, cfg):
    import ml_dtypes
    bf = ml_dtypes.bfloat16
    f32 = np.float32
    qkv_w = np.asarray(inputs['qkv_w'], f32)
    wq_t = np.ascontiguousarray(qkv_w[0:DIM].T).astype(bf)
    wk_t = np.ascontiguousarray(qkv_w[DIM:2 * DIM].T).astype(bf)
    wv_t = np.ascontiguousarray(qkv_w[2 * DIM:3 * DIM].T).astype(bf)
    proj_t = np.ascontiguousarray(np.asarray(inputs['proj_w'], f32).T).astype(bf)

    def diag_taps(wconv):
        w = np.asarray(wconv, f32).reshape(DIM, 9)
        out = np.zeros((NH, 9, HD, HD), f32)
        idx = np.arange(HD)
        for c in range(NH):
            for j in range(9):
                out[c, j, idx, idx] = w[c * HD:(c + 1) * HD, j]
        return out.astype(bf)

    def wvecs(wconv):
        w = np.asarray(wconv, f32).reshape(DIM, 9)
        return np.ascontiguousarray(w.reshape(NH, HD, 9))

    npix = cfg['H'] * W
    return dict(
        wq_t=wq_t, wk_t=wk_t, wv_t=wv_t, proj_t=proj_t,
        ident=np.eye(HD, dtype=f32).astype(bf),
        diag1q=diag_taps(inputs['sq_w']),
        diag1k=diag_taps(inputs['sk_w']),
        diag2=diag_taps(inputs['dwc_w']),
        wvec1q=wvecs(inputs['sq_w']),
        wvec1k=wvecs(inputs['sk_w']),
        wvec2=wvecs(inputs['dwc_w']),
        sq_b=np.asarray(inputs['sq_b'], f32).reshape(DIM, 1),
        sk_b=np.asarray(inputs['sk_b'], f32).reshape(DIM, 1),
        dwc_b=np.asarray(inputs['dwc_b'], f32).reshape(DIM, 1),
        proj_b=np.asarray(inputs['proj_b'], f32).reshape(DIM, 1),
        se_w1q=np.ascontiguousarray(
            np.asarray(inputs['cq_w1'], f32).transpose(0, 2, 1) / npix),
        se_b1q=np.asarray(inputs['cq_b1'], f32).reshape(NH, HD4, 1),
        se_w2q=np.ascontiguousarray(
            np.asarray(inputs['cq_w2'], f32).transpose(0, 2, 1)),
        se_b2q=np.asarray(inputs['cq_b2'], f32).reshape(NH, HD, 1),
        se_w1k=np.ascontiguousarray(
            np.asarray(inputs['ck_w1'], f32).transpose(0, 2, 1) / npix),
        se_b1k=np.asarray(inputs['ck_b1'], f32).reshape(NH, HD4, 1),
        se_w2k=np.ascontiguousarray(
            np.asarray(inputs['ck_w2'], f32).transpose(0, 2, 1)),
        se_b2k=np.asarray(inputs['ck_b2'], f32).reshape(NH, HD, 1),
    )


_CACHE = {}


def _get_compiled(cfg_key, cfg):
    if cfg_key not in _CACHE:
        _CACHE[cfg_key] = build_nc(cfg)
    return _CACHE[cfg_key]


def kernel(**inputs):
    import ml_dtypes
    from concourse import bass_utils
    cfg = default_cfg()
    nc = _get_compiled('main', cfg)
    w = prep_weights(inputs, cfg)
    x32 = np.asarray(inputs['x'], np.float32)
    x = x32.astype(ml_dtypes.bfloat16)
    BL = cfg['b_local']
    in_maps = []
    for core in range(N_CORES):
        m = dict(w)
        m['x'] = np.ascontiguousarray(x[core * BL:(core + 1) * BL])
        in_maps.append(m)
    res = bass_utils.run_bass_kernel_spmd(nc, in_maps, core_ids=list(range(N_CORES)))
    out = np.empty((B, DIM, H_FULL, W), np.float32)
    for core in range(N_CORES):
        out[core * BL:(core + 1) * BL] = res.results[core]['out']
    return out


# revision 17
# speedup vs baseline: 1.9811x; 1.9811x over previous
"""Trainium2 Bass kernel for nn_CASAtt_MultiHead_v1 (CAS attention block).

Reference computation (per sample):
    qkv = 1x1 conv (qkv_w) -> q, k, v                        [512, 56, 56] each
    q <- SE(dwconv3x3(q, sq_w, sq_b))   (per-head squeeze-excite)
    k <- SE(dwconv3x3(k, sk_w, sk_b))
    out = proj(dwconv3x3(q + k, dwc_w, dwc_b) * v) + proj_b + x

Distribution: data-parallel over batch, 2 samples per NeuronCore x 8 cores.

Layout: channels on partitions, 4 chunks of 128 (chunk == SE head).
GEMMs in bf16 on the PE (fp32 PSUM).  Depthwise 3x3 convs run either as
9 diagonal-matrix matmuls on the PE, or as scalar-MAC tap chains on the
vector engines over contiguous padded-flat slices (WP=59 so 5 of 9 tap
offsets are 4B-aligned and the DVE 2x bf16 mode engages; odd-parity taps
either run 1x STT on DVE or are offloaded as scaled copies to the
scalar/pool engines and accumulated with aligned 2x tensor_tensor adds).
Engine per (conv, head) is cfg-tunable to balance PE vs DVE vs ACT vs
POOL.  The residual (+x) is accumulated into the proj PSUM group via an
identity-matrix matmul, so the final drain is a single ACT activation.
m = s_q*dwq + s_k*dwk is built in-place in dwk; a single third conv runs
on m.  Mixed-dtype tensor-tensor DVE ops (psum f32 + bf16 operand)
produce NaN on hardware -- every tensor-tensor-class op keeps both
tensor operands the same dtype.
"""

import numpy as np

DIM = 512
NH = 4
HD = 128
HD4 = 32
B, H_FULL, W = 16, 56, 56
N_CORES = 8

TAPS = [(dy, dx) for dy in (-1, 0, 1) for dx in (-1, 0, 1)]


def default_cfg():
    # conv_assign: engine per (conv_id, head).  conv_id: 0=q, 1=k, 2=m.
    #   'pe'  diag matmuls on TensorE
    #   'dve' pure DVE tap chain (odd taps 1x STT)
    #   'dva' odd taps as ACT scaled copies + DVE 2x adds
    #   'dvp' odd taps as POOL scaled copies + DVE 2x adds
    assign = {
        (0, 0): 'dva', (0, 1): 'dva', (0, 2): 'dva', (0, 3): 'dva',
        (1, 0): 'dva', (1, 1): 'dva', (1, 2): 'dva', (1, 3): 'dva',
        (2, 0): 'dva', (2, 1): 'dva', (2, 2): 'pe', (2, 3): 'pe',
    }
    return dict(
        b_local=B // N_CORES,
        H=H_FULL,
        rows_per_tile=8,
        conv_assign=assign,
    )


def build_nc(cfg):
    """Build + compile the Bacc program for one core (SPMD across 8)."""
    import concourse.bass as bass
    import concourse.mybir as mybir
    import concourse.tile as tile
    from concourse import bacc
    from contextlib import ExitStack

    f32 = mybir.dt.float32
    bf16 = mybir.dt.bfloat16
    cdt = bf16
    gdt = bf16

    BL = cfg['b_local']
    H = cfg['H']
    TH = cfg['rows_per_tile']
    NT = H // TH
    assert NT * TH == H
    TN = TH * W
    HP, WP = H + 2, W + 3
    PADN = HP * WP
    TPAD = TH * WP
    AF = mybir.ActivationFunctionType
    AL = mybir.AluOpType
    assign = cfg['conv_assign']
    # taps ordered: even-parity (2x-capable) first, odds last; final tap
    # is strided (1x regardless) so it takes an odd slot
    evens = [j for j, (dy, dx) in enumerate(TAPS) if (dy * WP + dx) % 2 == 0]
    odds = [j for j, (dy, dx) in enumerate(TAPS) if (dy * WP + dx) % 2]

    nc = bacc.Bacc("TRN2", target_bir_lowering=False, debug=False,
                   enable_asserts=False, num_devices=N_CORES)

    # ---------------- DRAM I/O ----------------
    x_d = nc.dram_tensor("x", [BL, DIM, H, W], gdt, kind="ExternalInput").ap()
    out_d = nc.dram_tensor("out", [BL, DIM, H, W], f32, kind="ExternalOutput").ap()
    wq_d = nc.dram_tensor("wq_t", [DIM, DIM], gdt, kind="ExternalInput").ap()
    wk_d = nc.dram_tensor("wk_t", [DIM, DIM], gdt, kind="ExternalInput").ap()
    wv_d = nc.dram_tensor("wv_t", [DIM, DIM], gdt, kind="ExternalInput").ap()
    wp_d = nc.dram_tensor("proj_t", [DIM, DIM], gdt, kind="ExternalInput").ap()
    ident_d = nc.dram_tensor("ident", [HD, HD], gdt, kind="ExternalInput").ap()
    dg_d = [nc.dram_tensor(n, [NH, 9, HD, HD], cdt, kind="ExternalInput").ap()
            for n in ("diag1q", "diag1k", "diag2")]
    wv1_d = [nc.dram_tensor(n, [NH, HD, 9], f32, kind="ExternalInput").ap()
             for n in ("wvec1q", "wvec1k", "wvec2")]
    b1_d = [nc.dram_tensor(n, [DIM, 1], f32, kind="ExternalInput").ap()
            for n in ("sq_b", "sk_b", "dwc_b")]
    projb_d = nc.dram_tensor("proj_b", [DIM, 1], f32, kind="ExternalInput").ap()
    sew1_d = [nc.dram_tensor(n, [NH, HD, HD4], f32, kind="ExternalInput").ap()
              for n in ("se_w1q", "se_w1k")]
    seb1_d = [nc.dram_tensor(n, [NH, HD4, 1], f32, kind="ExternalInput").ap()
              for n in ("se_b1q", "se_b1k")]
    sew2_d = [nc.dram_tensor(n, [NH, HD4, HD], f32, kind="ExternalInput").ap()
              for n in ("se_w2q", "se_w2k")]
    seb2_d = [nc.dram_tensor(n, [NH, HD, 1], f32, kind="ExternalInput").ap()
              for n in ("se_b2q", "se_b2k")]

    with tile.TileContext(nc) as tc, ExitStack() as ctx:
        const = ctx.enter_context(tc.tile_pool(name="const", bufs=1))
        big = ctx.enter_context(tc.tile_pool(name="big", bufs=1))
        wpool = ctx.enter_context(tc.tile_pool(name="wpool", bufs=1))
        xpool = ctx.enter_context(tc.tile_pool(name="xpool", bufs=2))
        vpool = ctx.enter_context(tc.tile_pool(name="vpool", bufs=2))
        o2pool = ctx.enter_context(tc.tile_pool(name="o2pool", bufs=2))
        otpool = ctx.enter_context(tc.tile_pool(name="otpool", bufs=2))
        statpool = ctx.enter_context(tc.tile_pool(name="statpool", bufs=16))
        mmpool = ctx.enter_context(tc.tile_pool(name="mmpool", bufs=5, space="PSUM"))
        sepool = ctx.enter_context(tc.tile_pool(name="sepool", bufs=2, space="PSUM"))

        # ---------- persistent SBUF ----------
        # padded conv-domain buffers; 2-elem slop so padded-space tap reads
        # (offsets -WP-1 .. +WP+1) stay in bounds
        qpad = [big.tile([HD, PADN + 2], cdt, name=f"qpad{c}") for c in range(NH)]
        kpad = [big.tile([HD, PADN + 2], cdt, name=f"kpad{c}") for c in range(NH)]
        dwq = [big.tile([HD, PADN + 2], cdt, name=f"dwq{c}") for c in range(NH)]
        dwk = [big.tile([HD, PADN + 2], cdt, name=f"dwk{c}") for c in range(NH)]

        def pad3(t):
            return t[:, 1:1 + PADN].rearrange("p (h w) -> p h w", w=WP)

        qpad3, kpad3 = [pad3(t) for t in qpad], [pad3(t) for t in kpad]
        dwq3, dwk3 = [pad3(t) for t in dwq], [pad3(t) for t in dwk]

        # flat conv2 outputs for off-PE conv2 heads
        c2flat = {}
        for oc in range(NH):
            if assign[(2, oc)] != 'pe':
                c2flat[oc] = big.tile([HD, H * W], cdt, name=f"c2f{oc}")

        # ping-pong accumulators + odd-tap scratch for off-PE convs (pad
        # cells may hold stale garbage between uses; only interior is
        # consumed).  STT with dst==in1 runs ~5x slow on HW (in-place
        # read-write hazard), so accumulation alternates two buffers;
        # TT with dst==in0 is full speed and runs in place.
        acc_g = [big.tile([HD, PADN + 2], cdt, name=f"accg{i}") for i in range(2)]
        tmp_g = [big.tile([HD, PADN + 2], cdt, name=f"tmpg{i}") for i in range(2)]

        for tt in qpad + kpad + dwq + dwk + acc_g + tmp_g:
            nc.vector.memset(tt, 0.0)

        # small constants
        bias1 = [[const.tile([HD, 1], f32, name=f"b1_{ci}_{c}") for c in range(NH)]
                 for ci in range(3)]
        projb = [const.tile([HD, 1], f32, name=f"projb{c}") for c in range(NH)]
        for c in range(NH):
            sl = slice(c * HD, (c + 1) * HD)
            for ci in range(3):
                nc.sync.dma_start(bias1[ci][c], b1_d[ci][sl])
            nc.sync.dma_start(projb[c], projb_d[sl])
        sew1 = [[const.tile([HD, HD4], f32, name=f"sew1_{br}_{c}") for c in range(NH)]
                for br in range(2)]
        seb1 = [[const.tile([HD4, 1], f32, name=f"seb1_{br}_{c}") for c in range(NH)]
                for br in range(2)]
        sew2 = [[const.tile([HD4, HD], f32, name=f"sew2_{br}_{c}") for c in range(NH)]
                for br in range(2)]
        seb2 = [[const.tile([HD, 1], f32, name=f"seb2_{br}_{c}") for c in range(NH)]
                for br in range(2)]
        wvec = [[const.tile([HD, 9], f32, name=f"wvec_{ci}_{c}") for c in range(NH)]
                for ci in range(3)]
        for br in range(2):
            for c in range(NH):
                nc.sync.dma_start(sew1[br][c], sew1_d[br][c])
                nc.sync.dma_start(seb1[br][c], seb1_d[br][c])
                nc.sync.dma_start(sew2[br][c], sew2_d[br][c])
                nc.sync.dma_start(seb2[br][c], seb2_d[br][c])
        for ci in range(3):
            for c in range(NH):
                nc.sync.dma_start(wvec[ci][c], wv1_d[ci][c])

        # persistent GEMM weights (loaded once, reused across samples)
        ident = const.tile([HD, HD], gdt, name="ident")
        nc.sync.dma_start(ident, ident_d)

        def load_w(w_d, nm):
            w_sb = []
            for kc in range(NH):
                row = []
                for oc in range(NH):
                    wt = wpool.tile([HD, HD], gdt, name=f"{nm}{kc}_{oc}")
                    nc.sync.dma_start(wt, w_d[kc * HD:(kc + 1) * HD,
                                              oc * HD:(oc + 1) * HD])
                    row.append(wt)
                w_sb.append(row)
            return w_sb

        wq_sb = load_w(wq_d, "wq")
        wk_sb = load_w(wk_d, "wk")
        wv_sb = load_w(wv_d, "wv")
        wp_sb = load_w(wp_d, "wp")

        # persistent diag weights for PE-assigned conv heads
        dg_sb = {}
        for ci in range(3):
            for oc in range(NH):
                if assign[(ci, oc)] == 'pe':
                    dg_sb[(ci, oc)] = [
                        const.tile([HD, HD], cdt, name=f"dg{ci}_{oc}_{j}")
                        for j in range(9)]
                    for j in range(9):
                        nc.sync.dma_start(dg_sb[(ci, oc)][j], dg_d[ci][oc, j])

        def taps_flat_tile(tbuf, r0):
            """9 contiguous slices (full padded rows) for padded-space conv
            over output padded rows r0+1..r0+TH (tile granularity, PE)."""
            base = 1 + (r0 + 1) * WP
            return [tbuf[:, base + dy * WP + dx: base + dy * WP + dx + TPAD]
                    for (dy, dx) in TAPS]

        def tap_bounds(j):
            """Whole-chunk padded-flat bounds for tap j: covers padded rows
            1..H, start/count adjusted to even element parity."""
            dy, dx = TAPS[j]
            delta = dy * WP + dx
            lo, cnt = WP, H * WP
            if (1 + lo + delta) % 2:
                lo, cnt = lo + 1, cnt - 1
            if cnt % 2:
                cnt -= 1
            return lo, cnt, delta

        # canonical aligned superset of the interior for 2x TT accumulate
        # (element start 1+CLO = 60 is 4B-aligned; cells outside the
        # interior get garbage, which is never read)
        CLO, CCNT = WP, H * WP

        def conv_head(ci, oc, srcb, src3, dst3_or_flat, b, stats, eng,
                     want_stats):
            """One depthwise-conv head.  dst3_or_flat: 3-d padded view
            (conv1) or flat [HD, H*W] tile (conv2)."""
            bias = bias1[ci][oc]
            wv_ = wvec[ci][oc]
            sfx = f"{b}_{ci}_{oc}"
            if eng == 'pe':
                return  # handled separately (emission order differs)
            # evens: first with bias via TS, rest STT ping-ponging the
            # two accumulators (2x mode; dst != in1 keeps full speed)
            cur, oth = 0, 1
            j0 = evens[0]
            lo, cnt, delta = tap_bounds(j0)
            nc.vector.tensor_scalar(
                acc_g[cur][:, 1 + lo:1 + lo + cnt],
                srcb[:, 1 + lo + delta:1 + lo + delta + cnt],
                wv_[:, j0:j0 + 1], bias, AL.mult, AL.add)
            for jj in evens[1:]:
                lo, cnt, delta = tap_bounds(jj)
                nc.vector.scalar_tensor_tensor(
                    acc_g[oth][:, 1 + lo:1 + lo + cnt],
                    srcb[:, 1 + lo + delta:1 + lo + delta + cnt],
                    wv_[:, jj:jj + 1],
                    acc_g[cur][:, 1 + lo:1 + lo + cnt],
                    AL.mult, AL.add)
                cur, oth = oth, cur
            # odd taps (all but the final one): first len(tmp_g) go as
            # ACT scaled copies + aligned 2x in-place TT adds, the rest
            # as 1x ping-pong STT on DVE
            n_off = len(tmp_g) if eng == 'dva' else 0
            for i, jj in enumerate(odds[:-1][:n_off]):
                lo, cnt, delta = tap_bounds(jj)
                t_sl = tmp_g[i][:, 1 + lo:1 + lo + cnt]
                src_sl = srcb[:, 1 + lo + delta:1 + lo + delta + cnt]
                nc.scalar.activation(t_sl, src_sl, AF.Copy,
                                     scale=wv_[:, jj:jj + 1])
            for i, jj in enumerate(odds[:-1][:n_off]):
                nc.vector.tensor_tensor(
                    acc_g[cur][:, 1 + CLO:1 + CLO + CCNT],
                    acc_g[cur][:, 1 + CLO:1 + CLO + CCNT],
                    tmp_g[i][:, 1 + CLO:1 + CLO + CCNT],
                    AL.add)
            for jj in odds[:-1][n_off:]:
                lo, cnt, delta = tap_bounds(jj)
                nc.vector.scalar_tensor_tensor(
                    acc_g[oth][:, 1 + lo:1 + lo + cnt],
                    srcb[:, 1 + lo + delta:1 + lo + delta + cnt],
                    wv_[:, jj:jj + 1],
                    acc_g[cur][:, 1 + lo:1 + lo + cnt],
                    AL.mult, AL.add)
                cur, oth = oth, cur
            # final odd tap: strided interior finalize
            j8 = odds[-1]
            dy, dx = TAPS[j8]
            acc3 = pad3(acc_g[cur])
            kw = dict(accum_out=stats[:, 0:1]) if want_stats else {}
            if ci < 2:
                dst = dst3_or_flat[:, 1:1 + H, 1:1 + W]
            else:
                dst = dst3_or_flat.rearrange("p (h w) -> p h w", w=W)
            nc.vector.scalar_tensor_tensor(
                dst,
                src3[:, 1 + dy:1 + dy + H, 1 + dx:1 + dx + W],
                wv_[:, j8:j8 + 1],
                acc3[:, 1:1 + H, 1:1 + W],
                AL.mult, AL.add, **kw)

        def pe_conv_head(ci, oc, srcb, dst3, b, stats):
            """PE diag-matmul conv head (conv1 only: writes padded dst3
            with bias + accum pooling)."""
            dgs = dg_sb[(ci, oc)]
            for t in range(NT):
                r0 = t * TH
                ps = mmpool.tile([HD, TPAD], f32, tag="mm",
                                 name=f"c1{b}_{ci}_{t}_{oc}")
                for j, v in enumerate(taps_flat_tile(srcb, r0)):
                    nc.tensor.matmul(ps, dgs[j], v, start=(j == 0),
                                     stop=(j == 8))
                nc.scalar.activation(
                    dst3[:, 1 + r0:1 + r0 + TH, 1:1 + W],
                    ps.rearrange("p (h w) -> p h w", w=WP)[:, :, 1:1 + W],
                    AF.Identity, bias=bias1[ci][oc],
                    accum_out=stats[:, t:t + 1])

        def emit_se(b, br, oc, stats, pooled_w, s_scale):
            """SE chain for one head: pooled stats -> sigmoid scale."""
            pooled = const.tile([HD, 1], f32, tag="pooled", bufs=4,
                                name=f"pool{b}_{br}_{oc}")
            nc.vector.tensor_reduce(pooled, stats[:, 0:pooled_w],
                                    mybir.AxisListType.X, AL.add)
            ps1 = sepool.tile([HD4, 1], f32, tag="se", name=f"se1_{b}_{br}_{oc}")
            nc.tensor.matmul(ps1, sew1[br][oc], pooled, start=True, stop=True)
            hvec = const.tile([HD4, 1], f32, tag="hvec", bufs=4,
                              name=f"h{b}_{br}_{oc}")
            nc.scalar.activation(hvec, ps1, AF.Relu, bias=seb1[br][oc])
            ps2 = sepool.tile([HD, 1], f32, tag="se", name=f"se2_{b}_{br}_{oc}")
            nc.tensor.matmul(ps2, sew2[br][oc], hvec, start=True, stop=True)
            s_sb = const.tile([HD, 1], f32, tag="s_scale", bufs=16,
                              name=f"s{b}_{br}_{oc}")
            nc.scalar.activation(s_sb, ps2, AF.Sigmoid, bias=seb2[br][oc])
            s_scale[b][br][oc] = s_sb

        s_scale = [[[None] * NH for _ in range(2)] for _ in range(BL)]
        stats_t = {}

        def phaseA(b):
            """q,k GEMMs for sample b -> qpad/kpad."""
            for br in range(2):
                w_sb = wq_sb if br == 0 else wk_sb
                p3 = qpad3 if br == 0 else kpad3
                for t in range(NT):
                    r0 = t * TH
                    xt = []
                    for kc in range(NH):
                        xx = xpool.tile([HD, TN], gdt, tag=f"x{kc}",
                                        name=f"xa{kc}_b{b}_{br}_{t}")
                        nc.sync.dma_start(
                            xx.rearrange("p (h w) -> p h w", w=W),
                            x_d[b, kc * HD:(kc + 1) * HD, r0:r0 + TH, :])
                        xt.append(xx)
                    for oc in range(NH):
                        ps = mmpool.tile([HD, TN], f32, tag="mm",
                                         name=f"g{b}_{br}_{t}_{oc}")
                        for kc in range(NH):
                            nc.tensor.matmul(ps, w_sb[kc][oc], xt[kc],
                                             start=(kc == 0),
                                             stop=(kc == NH - 1))
                        nc.scalar.copy(
                            p3[oc][:, 1 + r0:1 + r0 + TH, 1:1 + W],
                            ps.rearrange("p (h w) -> p h w", w=W))

        def phaseB_offpe(b):
            """Off-PE conv1 heads (SE chains emitted later: their PE
            matmuls must not block the next sample's GEMMs in the PE
            queue)."""
            for br in range(2):
                srcb = qpad if br == 0 else kpad
                src3 = qpad3 if br == 0 else kpad3
                dst3 = dwq3 if br == 0 else dwk3
                for oc in range(NH):
                    eng = assign[(br, oc)]
                    if eng == 'pe':
                        continue
                    stats = statpool.tile([HD, NT], f32, tag="stats",
                                          name=f"st{b}_{br}_{oc}")
                    conv_head(br, oc, srcb[oc], src3[oc], dst3[oc], b,
                              stats, eng, True)
                    stats_t[(b, br, oc)] = stats

        def phaseB_offpe_se(b):
            for br in range(2):
                for oc in range(NH):
                    if assign[(br, oc)] == 'pe':
                        continue
                    emit_se(b, br, oc, stats_t[(b, br, oc)], 1, s_scale)

        def phaseB_pe(b):
            """PE conv1 heads + their SE chains."""
            for br in range(2):
                srcb = qpad if br == 0 else kpad
                dst3 = dwq3 if br == 0 else dwk3
                for oc in range(NH):
                    if assign[(br, oc)] != 'pe':
                        continue
                    stats = statpool.tile([HD, NT], f32, tag="stats",
                                          name=f"stp{b}_{br}_{oc}")
                    pe_conv_head(br, oc, srcb[oc], dst3[oc], b, stats)
                    emit_se(b, br, oc, stats, NT, s_scale)

        def phase15(b):
            """m = s_q*dwq + s_k*dwk -> dwk.  (STT dst==in1 is slow on
            HW, so stage the k term through an accumulator.)"""
            for oc in range(NH):
                scr = acc_g[oc % 2]
                nc.vector.tensor_scalar(scr, dwk[oc], s_scale[b][1][oc],
                                        None, AL.mult)
                nc.vector.scalar_tensor_tensor(dwk[oc], dwq[oc],
                                               s_scale[b][0][oc], scr,
                                               AL.mult, AL.add)

        def phaseC_offpe(b):
            """Off-PE conv2 heads -> c2flat."""
            for oc in range(NH):
                eng = assign[(2, oc)]
                if eng == 'pe':
                    continue
                conv_head(2, oc, dwk[oc], dwk3[oc], c2flat[oc], b,
                          None, eng, False)

        def phaseD(b):
            """Per row-tile: v GEMM, o2 = c2*v, proj GEMM + residual."""
            for t in range(NT):
                r0 = t * TH
                xt = []
                for kc in range(NH):
                    xx = xpool.tile([HD, TN], gdt, tag=f"x{kc}",
                                    name=f"xd{kc}_b{b}_{t}")
                    nc.sync.dma_start(
                        xx.rearrange("p (h w) -> p h w", w=W),
                        x_d[b, kc * HD:(kc + 1) * HD, r0:r0 + TH, :])
                    xt.append(xx)
                o2 = []
                for oc in range(NH):
                    ps = mmpool.tile([HD, TN], f32, tag="mm",
                                     name=f"v{b}_{t}_{oc}")
                    for kc in range(NH):
                        nc.tensor.matmul(ps, wv_sb[kc][oc], xt[kc],
                                         start=(kc == 0), stop=(kc == NH - 1))
                    vv = vpool.tile([HD, TN], gdt, tag=f"vt{oc}",
                                    name=f"vt{oc}_b{b}_{t}")
                    nc.scalar.copy(vv, ps)
                    if assign[(2, oc)] == 'pe':
                        # conv2 on PE for this head, this tile
                        ps2 = mmpool.tile([HD, TPAD], f32, tag="mm",
                                          name=f"c2{b}_{t}_{oc}")
                        for j, v in enumerate(taps_flat_tile(dwk[oc], r0)):
                            nc.tensor.matmul(ps2, dg_sb[(2, oc)][j], v,
                                             start=(j == 0), stop=(j == 8))
                        c2t = o2pool.tile([HD, TN], gdt, tag=f"c2t{oc}",
                                          name=f"c2t{oc}_b{b}_{t}")
                        nc.scalar.activation(
                            c2t.rearrange("p (h w) -> p h w", w=W),
                            ps2.rearrange("p (h w) -> p h w",
                                          w=WP)[:, :, 1:1 + W],
                            AF.Identity, bias=bias1[2][oc])
                        c2_sl = c2t
                    else:
                        c2_sl = c2flat[oc][:, r0 * W:r0 * W + TN]
                    oo = o2pool.tile([HD, TN], gdt, tag=f"o2_{oc}",
                                     name=f"o2_{oc}_b{b}_{t}")
                    nc.vector.tensor_mul(oo, c2_sl, vv)
                    o2.append(oo)
                for oc in range(NH):
                    ps = mmpool.tile([HD, TN], f32, tag="mm",
                                     name=f"p{b}_{t}_{oc}")
                    for kc in range(NH):
                        nc.tensor.matmul(ps, wp_sb[kc][oc], o2[kc],
                                         start=(kc == 0), stop=False)
                    # residual: accumulate x via identity matmul
                    nc.tensor.matmul(ps, ident, xt[oc], start=False,
                                     stop=True)
                    ot = otpool.tile([HD, TN], f32, tag="ot", bufs=3,
                                     name=f"ot{oc}_b{b}_{t}")
                    nc.scalar.activation(ot, ps, AF.Identity, bias=projb[oc])
                    nc.sync.dma_start(
                        out_d[b, oc * HD:(oc + 1) * HD, r0:r0 + TH, :],
                        ot.rearrange("p (h w) -> p h w", w=W))

        # ---- emission schedule: overlap sample b's vector-engine conv
        # phase with sample b+1's PE GEMM phase ----
        phaseA(0)
        for b in range(BL):
            phaseB_offpe(b)       # DVE/ACT/POOL conv1
            phaseB_pe(b)          # PE conv1 share + SE
            if b + 1 < BL:
                phaseA(b + 1)     # next sample's GEMMs fill the PE
            phaseB_offpe_se(b)
            phase15(b)
            phaseC_offpe(b)
            phaseD(b)

    nc.compile()
    return nc


# ---------------------------------------------------------------------------
# host-side weight prep
# ---------------------------------------------------------------------------

def prep_weights(inputs, cfg):
    import ml_dtypes
    bf = ml_dtypes.bfloat16
    f32 = np.float32
    qkv_w = np.asarray(inputs['qkv_w'], f32)
    wq_t = np.ascontiguousarray(qkv_w[0:DIM].T).astype(bf)
    wk_t = np.ascontiguousarray(qkv_w[DIM:2 * DIM].T).astype(bf)
    wv_t = np.ascontiguousarray(qkv_w[2 * DIM:3 * DIM].T).astype(bf)
    proj_t = np.ascontiguousarray(np.asarray(inputs['proj_w'], f32).T).astype(bf)

    def diag_taps(wconv):
        w = np.asarray(wconv, f32).reshape(DIM, 9)
        out = np.zeros((NH, 9, HD, HD), f32)
        idx = np.arange(HD)
        for c in range(NH):
            for j in range(9):
                out[c, j, idx, idx] = w[c * HD:(c + 1) * HD, j]
        return out.astype(bf)

    def wvecs(wconv):
        w = np.asarray(wconv, f32).reshape(DIM, 9)
        return np.ascontiguousarray(w.reshape(NH, HD, 9))

    npix = cfg['H'] * W
    return dict(
        wq_t=wq_t, wk_t=wk_t, wv_t=wv_t, proj_t=proj_t,
        ident=np.eye(HD, dtype=f32).astype(bf),
        diag1q=diag_taps(inputs['sq_w']),
        diag1k=diag_taps(inputs['sk_w']),
        diag2=diag_taps(inputs['dwc_w']),
        wvec1q=wvecs(inputs['sq_w']),
        wvec1k=wvecs(inputs['sk_w']),
        wvec2=wvecs(inputs['dwc_w']),
        sq_b=np.asarray(inputs['sq_b'], f32).reshape(DIM, 1),
        sk_b=np.asarray(inputs['sk_b'], f32).reshape(DIM, 1),
        dwc_b=np.asarray(inputs['dwc_b'], f32).reshape(DIM, 1),
        proj_b=np.asarray(inputs['proj_b'], f32).reshape(DIM, 1),
        se_w1q=np.ascontiguousarray(
            np.asarray(inputs['cq_w1'], f32).transpose(0, 2, 1) / npix),
        se_b1q=np.asarray(inputs['cq_b1'], f32).reshape(NH, HD4, 1),
        se_w2q=np.ascontiguousarray(
            np.asarray(inputs['cq_w2'], f32).transpose(0, 2, 1)),
        se_b2q=np.asarray(inputs['cq_b2'], f32).reshape(NH, HD, 1),
        se_w1k=np.ascontiguousarray(
            np.asarray(inputs['ck_w1'], f32).transpose(0, 2, 1) / npix),
        se_b1k=np.asarray(inputs['ck_b1'], f32).reshape(NH, HD4, 1),
        se_w2k=np.ascontiguousarray(
            np.asarray(inputs['ck_w2'], f32).transpose(0, 2, 1)),
        se_b2k=np.asarray(inputs['ck_b2'], f32).reshape(NH, HD, 1),
    )


_CACHE = {}


def _get_compiled(cfg_key, cfg):
    if cfg_key not in _CACHE:
        _CACHE[cfg_key] = build_nc(cfg)
    return _CACHE[cfg_key]


def kernel(**inputs):
    import ml_dtypes
    from concourse import bass_utils
    cfg = default_cfg()
    nc = _get_compiled('main', cfg)
    w = prep_weights(inputs, cfg)
    x32 = np.asarray(inputs['x'], np.float32)
    x = x32.astype(ml_dtypes.bfloat16)
    BL = cfg['b_local']
    in_maps = []
    for core in range(N_CORES):
        m = dict(w)
        m['x'] = np.ascontiguousarray(x[core * BL:(core + 1) * BL])
        in_maps.append(m)
    res = bass_utils.run_bass_kernel_spmd(nc, in_maps, core_ids=list(range(N_CORES)))
    out = np.empty((B, DIM, H_FULL, W), np.float32)
    for core in range(N_CORES):
        out[core * BL:(core + 1) * BL] = res.results[core]['out']
    return out


# revision 53
# speedup vs baseline: 2.9655x; 1.4969x over previous
"""Trainium2 Bass kernel for nn_CASAtt_MultiHead_v1 (CAS attention block).

Reference computation (per sample):
    qkv = 1x1 conv (qkv_w) -> q, k, v                        [512, 56, 56] each
    q <- SE(dwconv3x3(q, sq_w, sq_b))   (per-head squeeze-excite)
    k <- SE(dwconv3x3(k, sk_w, sk_b))
    out = proj(dwconv3x3(q + k, dwc_w, dwc_b) * v) + proj_b + x

Distribution: data-parallel over batch, 2 samples per NeuronCore x 8 cores.

Layout: channels on partitions, 4 chunks of 128 (chunk == SE head).
GEMMs in bf16 on the PE (fp32 PSUM).  Depthwise 3x3 convs run either as
9 diagonal-matrix matmuls on the PE, or as scalar-MAC tap chains on the
vector engines over contiguous padded-flat slices (WP=59 so 5 of 9 tap
offsets are 4B-aligned and the DVE 2x bf16 mode engages; odd-parity taps
either run 1x STT on DVE or are offloaded as scaled copies to the
scalar/pool engines and accumulated with aligned 2x tensor_tensor adds).
Engine per (conv, head) is cfg-tunable to balance PE vs DVE vs ACT vs
POOL.  The residual (+x) is accumulated into the proj PSUM group via an
identity-matrix matmul, so the final drain is a single ACT activation.
m = s_q*dwq + s_k*dwk is built in-place in dwk; a single third conv runs
on m.  Mixed-dtype tensor-tensor DVE ops (psum f32 + bf16 operand)
produce NaN on hardware -- every tensor-tensor-class op keeps both
tensor operands the same dtype.
"""

import numpy as np

DIM = 512
NH = 4
HD = 128
HD4 = 32
B, H_FULL, W = 16, 56, 56
N_CORES = 8

TAPS = [(dy, dx) for dy in (-1, 0, 1) for dx in (-1, 0, 1)]


def default_cfg():
    # conv_assign: engine per (conv_id, head).  conv_id: 0=q, 1=k, 2=m.
    #   'pe'  diag matmuls on TensorE
    #   'dve' pure DVE tap chain (odd taps 1x STT)
    #   'dva' odd taps as ACT scaled copies + DVE 2x adds
    #   'dvp' odd taps as POOL scaled copies + DVE 2x adds
    # int value = alpha: number of tap scale-copies offloaded to ACT
    # (rest run as 4x TS on DVE); 'pe' = diag matmuls on TensorE.
    # Keys (ci, oc) give the base assignment; (b, ci, oc) overrides for
    # one sample -- the last sample shifts heads onto the PE, which has
    # no next-sample GEMM to keep it busy during the conv phase.
    # NOTE: alpha must stay <= len(tmpA) (3): with more ACT copies than
    # scratch buffers, a later copy overwrites a buffer before the TT
    # that consumes the earlier one (emission order: copies then TTs)
    assign = {
        (0, 0): 3, (0, 1): 3, (0, 2): 3, (0, 3): 'pe',
        (1, 0): 3, (1, 1): 3, (1, 2): 'pe', (1, 3): 'pe',
        (2, 0): 'pe', (2, 1): 'pe', (2, 2): 3, (2, 3): 'pe',
        (0, 0, 2): 'pe',
        (1, 1, 1): 'pe',
    }
    return dict(
        b_local=B // N_CORES,
        H=H_FULL,
        rows_per_tile=8,
        conv_assign=assign,
    )


def build_nc(cfg):
    """Build + compile the Bacc program for one core (SPMD across 8)."""
    import concourse.bass as bass
    import concourse.mybir as mybir
    import concourse.tile as tile
    from concourse import bacc
    from contextlib import ExitStack

    f32 = mybir.dt.float32
    bf16 = mybir.dt.bfloat16
    cdt = bf16
    gdt = bf16

    BL = cfg['b_local']
    H = cfg['H']
    TH = cfg['rows_per_tile']
    NT = H // TH
    assert NT * TH == H
    TN = TH * W
    HP, WP = H + 2, W + 3
    PADN = HP * WP
    TPAD = TH * WP
    AF = mybir.ActivationFunctionType
    AL = mybir.AluOpType
    assign = cfg['conv_assign']

    def a_of(b, ci, oc):
        return assign.get((b, ci, oc), assign[(ci, oc)])

    nc = bacc.Bacc("TRN2", target_bir_lowering=False, debug=False,
                   enable_asserts=False, num_devices=N_CORES)

    # ---------------- DRAM I/O ----------------
    x_d = nc.dram_tensor("x", [BL, DIM, H, W], gdt, kind="ExternalInput").ap()
    out_d = nc.dram_tensor("out", [BL, DIM, H, W], f32, kind="ExternalOutput").ap()
    wq_d = nc.dram_tensor("wq_t", [DIM, DIM], gdt, kind="ExternalInput").ap()
    wk_d = nc.dram_tensor("wk_t", [DIM, DIM], gdt, kind="ExternalInput").ap()
    wv_d = nc.dram_tensor("wv_t", [DIM, DIM], gdt, kind="ExternalInput").ap()
    wp_d = nc.dram_tensor("proj_t", [DIM, DIM], gdt, kind="ExternalInput").ap()
    ident_d = nc.dram_tensor("ident", [HD, HD], gdt, kind="ExternalInput").ap()
    dg_d = [nc.dram_tensor(n, [NH, 9, HD, HD], cdt, kind="ExternalInput").ap()
            for n in ("diag1q", "diag1k", "diag2")]
    wv1_d = [nc.dram_tensor(n, [NH, HD, 9], f32, kind="ExternalInput").ap()
             for n in ("wvec1q", "wvec1k", "wvec2")]
    b1_d = [nc.dram_tensor(n, [DIM, 1], f32, kind="ExternalInput").ap()
            for n in ("sq_b", "sk_b", "dwc_b")]
    projb_d = nc.dram_tensor("proj_b", [DIM, 1], f32, kind="ExternalInput").ap()
    sew1_d = [nc.dram_tensor(n, [NH, HD, HD4], f32, kind="ExternalInput").ap()
              for n in ("se_w1q", "se_w1k")]
    seb1_d = [nc.dram_tensor(n, [NH, HD4, 1], f32, kind="ExternalInput").ap()
              for n in ("se_b1q", "se_b1k")]
    sew2_d = [nc.dram_tensor(n, [NH, HD4, HD], f32, kind="ExternalInput").ap()
              for n in ("se_w2q", "se_w2k")]
    seb2_d = [nc.dram_tensor(n, [NH, HD, 1], f32, kind="ExternalInput").ap()
              for n in ("se_b2q", "se_b2k")]

    with tile.TileContext(nc) as tc, ExitStack() as ctx:
        const = ctx.enter_context(tc.tile_pool(name="const", bufs=1))
        big = ctx.enter_context(tc.tile_pool(name="big", bufs=1))
        wpool = ctx.enter_context(tc.tile_pool(name="wpool", bufs=1))
        xpool = ctx.enter_context(tc.tile_pool(name="xpool", bufs=2))
        vpool = ctx.enter_context(tc.tile_pool(name="vpool", bufs=2))
        o2pool = ctx.enter_context(tc.tile_pool(name="o2pool", bufs=2))
        otpool = ctx.enter_context(tc.tile_pool(name="otpool", bufs=2))
        statpool = ctx.enter_context(tc.tile_pool(name="statpool", bufs=10))
        mmpool = ctx.enter_context(tc.tile_pool(name="mmpool", bufs=6, space="PSUM"))
        sepool = ctx.enter_context(tc.tile_pool(name="sepool", bufs=2, space="PSUM"))

        # ---------- persistent SBUF ----------
        # padded conv-domain buffers; 2-elem slop so padded-space tap reads
        # (offsets -WP-1 .. +WP+1) stay in bounds
        qpad = [big.tile([HD, PADN + 2], cdt, name=f"qpad{c}") for c in range(NH)]
        kpad = [big.tile([HD, PADN + 2], cdt, name=f"kpad{c}") for c in range(NH)]
        dwq = [big.tile([HD, PADN + 2], cdt, name=f"dwq{c}") for c in range(NH)]
        dwk = [big.tile([HD, PADN + 2], cdt, name=f"dwk{c}") for c in range(NH)]

        def pad3(t):
            return t[:, 1:1 + PADN].rearrange("p (h w) -> p h w", w=WP)

        qpad3, kpad3 = [pad3(t) for t in qpad], [pad3(t) for t in kpad]
        dwq3, dwk3 = [pad3(t) for t in dwq], [pad3(t) for t in dwk]

        # off-PE conv machinery: per tap, a full-tile scaled copy of the
        # source (TS on DVE runs 4x; ACT copy offloads it), then an
        # in-place TT add (2x, shift rides the second operand's slice --
        # measured insensitive to operand alignment).  STT is 1x always;
        # never use it on the big buffers.
        acc_g = big.tile([HD, PADN + 2], cdt, name="accg")
        tmpA = [big.tile([HD, PADN + 2], cdt, name=f"tmpA{i}") for i in range(3)]
        tmpS = big.tile([HD, PADN + 2], cdt, name="tmpS")

        # pad-zero invariant for the conv buffers: only the cells outside
        # the interior need zeroing (interiors are written before read),
        # so three small memsets per tile instead of a full-tile sweep.
        # The scratch tiles (acc_g/tmpA/tmpS) need no init at all.
        for tt in qpad + kpad + dwq + dwk:
            nc.vector.memset(tt[:, 0:1 + WP + 1], 0.0)
            nc.vector.memset(tt[:, 1 + 57 * WP:PADN + 2], 0.0)
            nc.vector.memset(
                tt[:, WP - 1:WP - 1 + 57 * WP].rearrange(
                    "p (h w) -> p h w", w=WP)[:, :, 0:3], 0.0)

        # small constants
        bias1 = [[const.tile([HD, 1], f32, name=f"b1_{ci}_{c}") for c in range(NH)]
                 for ci in range(3)]
        projb = [const.tile([HD, 1], f32, name=f"projb{c}") for c in range(NH)]
        for c in range(NH):
            sl = slice(c * HD, (c + 1) * HD)
            for ci in range(3):
                nc.sync.dma_start(bias1[ci][c], b1_d[ci][sl])
            nc.sync.dma_start(projb[c], projb_d[sl])
        wvec = [[const.tile([HD, 9], f32, name=f"wvec_{ci}_{c}") for c in range(NH)]
                for ci in range(3)]
        for ci in range(3):
            for c in range(NH):
                nc.sync.dma_start(wvec[ci][c], wv1_d[ci][c])

        sew1 = [[const.tile([HD, HD4], f32, name=f"sew1_{br}_{c}") for c in range(NH)]
                for br in range(2)]
        seb1 = [[const.tile([HD4, 1], f32, name=f"seb1_{br}_{c}") for c in range(NH)]
                for br in range(2)]
        sew2 = [[const.tile([HD4, HD], f32, name=f"sew2_{br}_{c}") for c in range(NH)]
                for br in range(2)]
        seb2 = [[const.tile([HD, 1], f32, name=f"seb2_{br}_{c}") for c in range(NH)]
                for br in range(2)]

        def load_se_consts():
            for br in range(2):
                for c in range(NH):
                    nc.sync.dma_start(sew1[br][c], sew1_d[br][c])
                    nc.sync.dma_start(seb1[br][c], seb1_d[br][c])
                    nc.sync.dma_start(sew2[br][c], sew2_d[br][c])
                    nc.sync.dma_start(seb2[br][c], seb2_d[br][c])

        # persistent GEMM weights (loaded once, reused across samples;
        # DMA emission deferred into the schedule so the first sample's
        # x tiles aren't queued behind the full weight set)
        ident = const.tile([HD, HD], gdt, name="ident")

        def load_w(w_d, nm):
            w_sb = []
            for kc in range(NH):
                row = []
                for oc in range(NH):
                    wt = wpool.tile([HD, HD], gdt, name=f"{nm}{kc}_{oc}")
                    nc.sync.dma_start(wt, w_d[kc * HD:(kc + 1) * HD,
                                              oc * HD:(oc + 1) * HD])
                    row.append(wt)
                w_sb.append(row)
            return w_sb

        wq_sb = wk_sb = wv_sb = wp_sb = None
        dg_sb = {}

        def load_dg():
            for ci in range(3):
                for oc in range(NH):
                    if any(a_of(b, ci, oc) == 'pe' for b in range(BL)):
                        dg_sb[(ci, oc)] = [
                            const.tile([HD, HD], cdt, name=f"dg{ci}_{oc}_{j}")
                            for j in range(9)]
                        for j in range(9):
                            nc.sync.dma_start(dg_sb[(ci, oc)][j],
                                              dg_d[ci][oc, j])

        def taps_flat_tile(tbuf, r0):
            """9 contiguous slices (full padded rows) for padded-space conv
            over output padded rows r0+1..r0+TH (tile granularity, PE)."""
            base = 1 + (r0 + 1) * WP
            return [tbuf[:, base + dy * WP + dx: base + dy * WP + dx + TPAD]
                    for (dy, dx) in TAPS]

        # accumulate window: elements [ALO, ALO+ACNT) -- 4B-aligned start,
        # superset of the interior; src windows [ALO+d, ALO+ACNT+d) stay
        # in bounds for every tap offset d in [-60, 60]
        ALO, ACNT = WP + 1, H * WP
        CPN = PADN + 2  # full scale-copy window

        def conv_head(ci, oc, srcb, dst3, b, stats, alpha):
            """One off-PE depthwise-conv head.  Per tap: full-tile scaled
            copy (first `alpha` non-base taps on ACT, rest as 4x TS on
            DVE), then 2x in-place TT add with the shift in the second
            operand's slice.  Final strided extraction on ACT adds the
            pooling accum.  Bias rides the base-tap TS."""
            assert alpha <= len(tmpA)
            bias = bias1[ci][oc]
            wv_ = wvec[ci][oc]
            deltas = [dy * WP + dx for (dy, dx) in TAPS]
            act_taps = list(range(1, 1 + alpha))
            ts_taps = list(range(1 + alpha, 9))
            # conv2 accumulates straight into dwq[oc] (dead after
            # phase15): interior is all phaseD reads, in-window pad cols
            # are re-zeroed before the next sample's phase15
            acc = dwq[oc] if ci == 2 else acc_g
            for i, j in enumerate(act_taps):
                nc.scalar.activation(tmpA[i % 3][:, 0:CPN],
                                     srcb[:, 0:CPN], AF.Copy,
                                     scale=wv_[:, j:j + 1])
            d0 = deltas[0]
            nc.vector.tensor_scalar(
                acc[:, ALO:ALO + ACNT],
                srcb[:, ALO + d0:ALO + d0 + ACNT],
                wv_[:, 0:1], bias, AL.mult, AL.add)
            for j in ts_taps:
                nc.vector.tensor_scalar(tmpS[:, 0:CPN], srcb[:, 0:CPN],
                                        wv_[:, j:j + 1], None, AL.mult)
                d = deltas[j]
                nc.vector.tensor_tensor(
                    acc[:, ALO:ALO + ACNT],
                    acc[:, ALO:ALO + ACNT],
                    tmpS[:, ALO + d:ALO + d + ACNT], AL.add)
            for i, j in enumerate(act_taps):
                d = deltas[j]
                nc.vector.tensor_tensor(
                    acc[:, ALO:ALO + ACNT],
                    acc[:, ALO:ALO + ACNT],
                    tmpA[i % 3][:, ALO + d:ALO + d + ACNT], AL.add)
            if ci == 2:
                return
            # strided interior extraction + pooling accum for conv1
            acc3 = pad3(acc_g)
            nc.scalar.activation(
                dst3[:, 1:1 + H, 1:1 + W],
                acc3[:, 1:1 + H, 1:1 + W],
                AF.Identity, bias=0.0, accum_out=stats[:, 0:1])

        def pe_conv_head(ci, oc, srcb, dst3, b, stats):
            """PE diag-matmul conv head (conv1 only: writes padded dst3
            with bias + accum pooling)."""
            dgs = dg_sb[(ci, oc)]
            for t in range(NT):
                r0 = t * TH
                ps = mmpool.tile([HD, TPAD], f32, tag="mm",
                                 name=f"c1{b}_{ci}_{t}_{oc}")
                for j, v in enumerate(taps_flat_tile(srcb, r0)):
                    nc.tensor.matmul(ps, dgs[j], v, start=(j == 0),
                                     stop=(j == 8))
                nc.scalar.activation(
                    dst3[:, 1 + r0:1 + r0 + TH, 1:1 + W],
                    ps.rearrange("p (h w) -> p h w", w=WP)[:, :, 1:1 + W],
                    AF.Identity, bias=bias1[ci][oc],
                    accum_out=stats[:, t:t + 1])

        def emit_se(b, br, oc, stats, pooled_w, s_scale):
            """SE chain for one head: pooled stats -> sigmoid scale."""
            pooled = const.tile([HD, 1], f32, tag="pooled", bufs=4,
                                name=f"pool{b}_{br}_{oc}")
            nc.vector.tensor_reduce(pooled, stats[:, 0:pooled_w],
                                    mybir.AxisListType.X, AL.add)
            ps1 = sepool.tile([HD4, 1], f32, tag="se", name=f"se1_{b}_{br}_{oc}")
            nc.tensor.matmul(ps1, sew1[br][oc], pooled, start=True, stop=True)
            hvec = const.tile([HD4, 1], f32, tag="hvec", bufs=4,
                              name=f"h{b}_{br}_{oc}")
            nc.scalar.activation(hvec, ps1, AF.Relu, bias=seb1[br][oc])
            ps2 = sepool.tile([HD, 1], f32, tag="se", name=f"se2_{b}_{br}_{oc}")
            nc.tensor.matmul(ps2, sew2[br][oc], hvec, start=True, stop=True)
            s_sb = const.tile([HD, 1], f32, tag="s_scale", bufs=16,
                              name=f"s{b}_{br}_{oc}")
            nc.scalar.activation(s_sb, ps2, AF.Sigmoid, bias=seb2[br][oc])
            s_scale[b][br][oc] = s_sb

        s_scale = [[[None] * NH for _ in range(2)] for _ in range(BL)]
        stats_t = {}

        def phaseA(b, br):
            """One branch's GEMM for sample b -> qpad/kpad."""
            if True:
                w_sb = wq_sb if br == 0 else wk_sb
                p3 = qpad3 if br == 0 else kpad3
                for t in range(NT):
                    r0 = t * TH
                    xt = []
                    for kc in range(NH):
                        xx = xpool.tile([HD, TN], gdt, tag=f"x{kc}",
                                        name=f"xa{kc}_b{b}_{br}_{t}")
                        nc.sync.dma_start(
                            xx.rearrange("p (h w) -> p h w", w=W),
                            x_d[b, kc * HD:(kc + 1) * HD, r0:r0 + TH, :])
                        xt.append(xx)
                    for oc in range(NH):
                        ps = mmpool.tile([HD, TN], f32, tag="mm",
                                         name=f"g{b}_{br}_{t}_{oc}")
                        for kc in range(NH):
                            nc.tensor.matmul(ps, w_sb[kc][oc], xt[kc],
                                             start=(kc == 0),
                                             stop=(kc == NH - 1))
                        nc.scalar.copy(
                            p3[oc][:, 1 + r0:1 + r0 + TH, 1:1 + W],
                            ps.rearrange("p (h w) -> p h w", w=W))

        def phaseB(b, br, which):
            """conv1 heads for one branch; `which` selects 'pe' or
            off-PE heads so PE conv work can be emitted ahead of the
            next sample's GEMM in the (in-order) PE queue.  SE chains
            are emitted later."""
            srcb = qpad if br == 0 else kpad
            dst3 = dwq3 if br == 0 else dwk3
            for oc in range(NH):
                eng = a_of(b, br, oc)
                if (eng == 'pe') != (which == 'pe'):
                    continue
                stats = statpool.tile([HD, NT], f32, tag="stats",
                                      name=f"st{b}_{br}_{oc}")
                if eng == 'pe':
                    pe_conv_head(br, oc, srcb[oc], dst3[oc], b, stats)
                else:
                    conv_head(br, oc, srcb[oc], dst3[oc], b, stats, eng)
                stats_t[(b, br, oc)] = stats

        def phaseB_se(b):
            for br in range(2):
                for oc in range(NH):
                    pw = NT if a_of(b, br, oc) == 'pe' else 1
                    emit_se(b, br, oc, stats_t[(b, br, oc)], pw, s_scale)

        def phase15(b):
            """m = s_q*dwq + s_k*dwk -> dwk, all TS/TT (STT is 1x)."""
            if b > 0:
                # previous sample's conv2 dirtied dwq's in-window pad
                # cols (w in {0,57,58}, rows 1..56); re-zero before the
                # full-tile TS reads them.  Pad rows 0/57 sit outside
                # the accumulate window and stayed zero.
                for oc in range(NH):
                    v = dwq[oc][:, WP - 1:WP - 1 + 57 * WP].rearrange(
                        "p (h w) -> p h w", w=WP)[:, :, 0:3]
                    nc.gpsimd.memset(v, 0.0)
            for oc in range(NH):
                nc.vector.tensor_scalar(acc_g[:, 0:CPN], dwk[oc][:, 0:CPN],
                                        s_scale[b][1][oc], None, AL.mult)
                nc.vector.tensor_scalar(dwk[oc][:, 0:CPN], dwq[oc][:, 0:CPN],
                                        s_scale[b][0][oc], None, AL.mult)
                nc.vector.tensor_tensor(dwk[oc][:, 0:CPN], dwk[oc][:, 0:CPN],
                                        acc_g[:, 0:CPN], AL.add)

        def phaseC_offpe(b):
            """Off-PE conv2 heads: m -> dwq (dwq is dead after phase15;
            its pads are permanently zero, so phaseD can read the conv2
            result from dwq3's interior)."""
            for oc in range(NH):
                eng = a_of(b, 2, oc)
                if eng == 'pe':
                    continue
                conv_head(2, oc, dwk[oc], dwq3[oc], b, None, eng)

        def phaseD(b):
            """Per row-tile: v GEMM, o2 = c2*v, proj GEMM + residual."""
            for t in range(NT):
                r0 = t * TH
                xt = []
                for kc in range(NH):
                    xx = xpool.tile([HD, TN], gdt, tag=f"x{kc}",
                                    name=f"xd{kc}_b{b}_{t}")
                    nc.sync.dma_start(
                        xx.rearrange("p (h w) -> p h w", w=W),
                        x_d[b, kc * HD:(kc + 1) * HD, r0:r0 + TH, :])
                    xt.append(xx)
                o2 = []
                for oc in range(NH):
                    ps = mmpool.tile([HD, TN], f32, tag="mm",
                                     name=f"v{b}_{t}_{oc}")
                    for kc in range(NH):
                        nc.tensor.matmul(ps, wv_sb[kc][oc], xt[kc],
                                         start=(kc == 0), stop=(kc == NH - 1))
                    vv = vpool.tile([HD, TN], gdt, tag="vt", bufs=6,
                                    name=f"vt{oc}_b{b}_{t}")
                    nc.scalar.copy(vv, ps)
                    if a_of(b, 2, oc) == 'pe':
                        # conv2 on PE for this head, this tile
                        ps2 = mmpool.tile([HD, TPAD], f32, tag="mm",
                                          name=f"c2{b}_{t}_{oc}")
                        for j, v in enumerate(taps_flat_tile(dwk[oc], r0)):
                            nc.tensor.matmul(ps2, dg_sb[(2, oc)][j], v,
                                             start=(j == 0), stop=(j == 8))
                        c2t = o2pool.tile([HD, TN], gdt, tag="c2t", bufs=4,
                                          name=f"c2t{oc}_b{b}_{t}")
                        nc.scalar.activation(
                            c2t.rearrange("p (h w) -> p h w", w=W),
                            ps2.rearrange("p (h w) -> p h w",
                                          w=WP)[:, :, 1:1 + W],
                            AF.Identity, bias=bias1[2][oc])
                        c2_sl = c2t.rearrange("p (h w) -> p h w", w=W)
                    else:
                        c2_sl = dwq3[oc][:, 1 + r0:1 + r0 + TH, 1:1 + W]
                    oo = o2pool.tile([HD, TN], gdt, tag="o2", bufs=6,
                                     name=f"o2_{oc}_b{b}_{t}")
                    nc.vector.tensor_mul(oo.rearrange("p (h w) -> p h w", w=W),
                                         c2_sl,
                                         vv.rearrange("p (h w) -> p h w", w=W))
                    o2.append(oo)
                for oc in range(NH):
                    ps = mmpool.tile([HD, TN], f32, tag="mm",
                                     name=f"p{b}_{t}_{oc}")
                    for kc in range(NH):
                        nc.tensor.matmul(ps, wp_sb[kc][oc], o2[kc],
                                         start=(kc == 0), stop=False)
                    # residual: accumulate x via identity matmul
                    nc.tensor.matmul(ps, ident, xt[oc], start=False,
                                     stop=True)
                    ot = otpool.tile([HD, TN], f32, tag="ot", bufs=2,
                                     name=f"ot{oc}_b{b}_{t}")
                    nc.scalar.activation(ot, ps, AF.Identity, bias=projb[oc])
                    nc.sync.dma_start(
                        out_d[b, oc * HD:(oc + 1) * HD, r0:r0 + TH, :],
                        ot.rearrange("p (h w) -> p h w", w=W))

        # ---- emission schedule: per branch, sample b's conv heads free
        # qpad/kpad, then sample b+1's GEMM for that branch refills it,
        # keeping the PE busy while the vector engines run the convs.
        # Weight DMAs are interleaved so the DMA queue serves the first
        # GEMM's x tiles early; bulk weights ride behind them ----
        wq_sb = load_w(wq_d, "wq")
        phaseA(0, 0)
        wk_sb = load_w(wk_d, "wk")
        phaseA(0, 1)
        load_dg()
        wv_sb = load_w(wv_d, "wv")
        wp_sb = load_w(wp_d, "wp")
        nc.sync.dma_start(ident, ident_d)
        load_se_consts()
        for b in range(BL):
            phaseB(b, 0, 'off')   # conv1-q off-PE chains (DVE/ACT)
            phaseB(b, 0, 'pe')    # conv1-q PE heads
            phaseB(b, 1, 'pe')    # conv1-k PE heads: PE work that
                                  # covers the next q-GEMM's wait for
                                  # qpad to be freed by the q chains
            if b + 1 < BL:
                phaseA(b + 1, 0)  # next sample's q GEMM fills the PE
            phaseB(b, 1, 'off')   # conv1-k off-PE chains
            if b + 1 < BL:
                phaseA(b + 1, 1)
            phaseB_se(b)
            phase15(b)
            phaseC_offpe(b)
            phaseD(b)

    nc.compile()
    return nc


# ---------------------------------------------------------------------------
# host-side weight prep
# ---------------------------------------------------------------------------

def prep_weights(inputs, cfg):
    import ml_dtypes
    bf = ml_dtypes.bfloat16
    f32 = np.float32
    qkv_w = np.asarray(inputs['qkv_w'], f32)
    wq_t = np.ascontiguousarray(qkv_w[0:DIM].T).astype(bf)
    wk_t = np.ascontiguousarray(qkv_w[DIM:2 * DIM].T).astype(bf)
    wv_t = np.ascontiguousarray(qkv_w[2 * DIM:3 * DIM].T).astype(bf)
    proj_t = np.ascontiguousarray(np.asarray(inputs['proj_w'], f32).T).astype(bf)

    def diag_taps(wconv):
        w = np.asarray(wconv, f32).reshape(DIM, 9)
        out = np.zeros((NH, 9, HD, HD), f32)
        idx = np.arange(HD)
        for c in range(NH):
            for j in range(9):
                out[c, j, idx, idx] = w[c * HD:(c + 1) * HD, j]
        return out.astype(bf)

    def wvecs(wconv):
        w = np.asarray(wconv, f32).reshape(DIM, 9)
        return np.ascontiguousarray(w.reshape(NH, HD, 9))

    npix = cfg['H'] * W
    return dict(
        wq_t=wq_t, wk_t=wk_t, wv_t=wv_t, proj_t=proj_t,
        ident=np.eye(HD, dtype=f32).astype(bf),
        diag1q=diag_taps(inputs['sq_w']),
        diag1k=diag_taps(inputs['sk_w']),
        diag2=diag_taps(inputs['dwc_w']),
        wvec1q=wvecs(inputs['sq_w']),
        wvec1k=wvecs(inputs['sk_w']),
        wvec2=wvecs(inputs['dwc_w']),
        sq_b=np.asarray(inputs['sq_b'], f32).reshape(DIM, 1),
        sk_b=np.asarray(inputs['sk_b'], f32).reshape(DIM, 1),
        dwc_b=np.asarray(inputs['dwc_b'], f32).reshape(DIM, 1),
        proj_b=np.asarray(inputs['proj_b'], f32).reshape(DIM, 1),
        se_w1q=np.ascontiguousarray(
            np.asarray(inputs['cq_w1'], f32).transpose(0, 2, 1) / npix),
        se_b1q=np.asarray(inputs['cq_b1'], f32).reshape(NH, HD4, 1),
        se_w2q=np.ascontiguousarray(
            np.asarray(inputs['cq_w2'], f32).transpose(0, 2, 1)),
        se_b2q=np.asarray(inputs['cq_b2'], f32).reshape(NH, HD, 1),
        se_w1k=np.ascontiguousarray(
            np.asarray(inputs['ck_w1'], f32).transpose(0, 2, 1) / npix),
        se_b1k=np.asarray(inputs['ck_b1'], f32).reshape(NH, HD4, 1),
        se_w2k=np.ascontiguousarray(
            np.asarray(inputs['ck_w2'], f32).transpose(0, 2, 1)),
        se_b2k=np.asarray(inputs['ck_b2'], f32).reshape(NH, HD, 1),
    )


_CACHE = {}


def _get_compiled(cfg_key, cfg):
    if cfg_key not in _CACHE:
        _CACHE[cfg_key] = build_nc(cfg)
    return _CACHE[cfg_key]


def kernel(**inputs):
    import ml_dtypes
    from concourse import bass_utils
    cfg = default_cfg()
    nc = _get_compiled('main', cfg)
    w = prep_weights(inputs, cfg)
    x32 = np.asarray(inputs['x'], np.float32)
    x = x32.astype(ml_dtypes.bfloat16)
    BL = cfg['b_local']
    in_maps = []
    for core in range(N_CORES):
        m = dict(w)
        m['x'] = np.ascontiguousarray(x[core * BL:(core + 1) * BL])
        in_maps.append(m)
    res = bass_utils.run_bass_kernel_spmd(nc, in_maps, core_ids=list(range(N_CORES)))
    out = np.empty((B, DIM, H_FULL, W), np.float32)
    for core in range(N_CORES):
        out[core * BL:(core + 1) * BL] = res.results[core]['out']
    return out


# revision 57
# speedup vs baseline: 2.9930x; 1.0093x over previous
"""Trainium2 Bass kernel for nn_CASAtt_MultiHead_v1 (CAS attention block).

Reference computation (per sample):
    qkv = 1x1 conv (qkv_w) -> q, k, v                        [512, 56, 56] each
    q <- SE(dwconv3x3(q, sq_w, sq_b))   (per-head squeeze-excite)
    k <- SE(dwconv3x3(k, sk_w, sk_b))
    out = proj(dwconv3x3(q + k, dwc_w, dwc_b) * v) + proj_b + x

Distribution: data-parallel over batch, 2 samples per NeuronCore x 8 cores.

Layout: channels on partitions, 4 chunks of 128 (chunk == SE head).
GEMMs in bf16 on the PE (fp32 PSUM).  Depthwise 3x3 convs run either as
9 diagonal-matrix matmuls on the PE, or as scalar-MAC tap chains on the
vector engines over contiguous padded-flat slices (WP=59 so 5 of 9 tap
offsets are 4B-aligned and the DVE 2x bf16 mode engages; odd-parity taps
either run 1x STT on DVE or are offloaded as scaled copies to the
scalar/pool engines and accumulated with aligned 2x tensor_tensor adds).
Engine per (conv, head) is cfg-tunable to balance PE vs DVE vs ACT vs
POOL.  The residual (+x) is accumulated into the proj PSUM group via an
identity-matrix matmul, so the final drain is a single ACT activation.
m = s_q*dwq + s_k*dwk is built in-place in dwk; a single third conv runs
on m.  Mixed-dtype tensor-tensor DVE ops (psum f32 + bf16 operand)
produce NaN on hardware -- every tensor-tensor-class op keeps both
tensor operands the same dtype.
"""

import numpy as np

DIM = 512
NH = 4
HD = 128
HD4 = 32
B, H_FULL, W = 16, 56, 56
N_CORES = 8

TAPS = [(dy, dx) for dy in (-1, 0, 1) for dx in (-1, 0, 1)]


def default_cfg():
    # conv_assign: engine per (conv_id, head).  conv_id: 0=q, 1=k, 2=m.
    #   'pe'  diag matmuls on TensorE
    #   'dve' pure DVE tap chain (odd taps 1x STT)
    #   'dva' odd taps as ACT scaled copies + DVE 2x adds
    #   'dvp' odd taps as POOL scaled copies + DVE 2x adds
    # int value = alpha: number of tap scale-copies offloaded to ACT
    # (rest run as 4x TS on DVE); 'pe' = diag matmuls on TensorE.
    # Keys (ci, oc) give the base assignment; (b, ci, oc) overrides for
    # one sample -- the last sample shifts heads onto the PE, which has
    # no next-sample GEMM to keep it busy during the conv phase.
    # NOTE: alpha must stay <= len(tmpA) (3): with more ACT copies than
    # scratch buffers, a later copy overwrites a buffer before the TT
    # that consumes the earlier one (emission order: copies then TTs)
    assign = {
        (0, 0): 3, (0, 1): 3, (0, 2): 3, (0, 3): 'pe',
        (1, 0): 3, (1, 1): 3, (1, 2): 'pe', (1, 3): 'pe',
        (2, 0): 'pe', (2, 1): 'pe', (2, 2): 3, (2, 3): 'pe',
        (0, 0, 2): 'pe',
        (1, 1, 1): 'pe',
        (0, 1, 2): 3,
    }
    return dict(
        b_local=B // N_CORES,
        H=H_FULL,
        rows_per_tile=8,
        conv_assign=assign,
    )


def build_nc(cfg):
    """Build + compile the Bacc program for one core (SPMD across 8)."""
    import concourse.bass as bass
    import concourse.mybir as mybir
    import concourse.tile as tile
    from concourse import bacc
    from contextlib import ExitStack

    f32 = mybir.dt.float32
    bf16 = mybir.dt.bfloat16
    cdt = bf16
    gdt = bf16

    BL = cfg['b_local']
    H = cfg['H']
    TH = cfg['rows_per_tile']
    NT = H // TH
    assert NT * TH == H
    TN = TH * W
    HP, WP = H + 2, W + 3
    PADN = HP * WP
    TPAD = TH * WP
    AF = mybir.ActivationFunctionType
    AL = mybir.AluOpType
    assign = cfg['conv_assign']

    def a_of(b, ci, oc):
        return assign.get((b, ci, oc), assign[(ci, oc)])

    nc = bacc.Bacc("TRN2", target_bir_lowering=False, debug=False,
                   enable_asserts=False, num_devices=N_CORES)

    # ---------------- DRAM I/O ----------------
    x_d = nc.dram_tensor("x", [BL, DIM, H, W], gdt, kind="ExternalInput").ap()
    out_d = nc.dram_tensor("out", [BL, DIM, H, W], f32, kind="ExternalOutput").ap()
    wq_d = nc.dram_tensor("wq_t", [DIM, DIM], gdt, kind="ExternalInput").ap()
    wk_d = nc.dram_tensor("wk_t", [DIM, DIM], gdt, kind="ExternalInput").ap()
    wv_d = nc.dram_tensor("wv_t", [DIM, DIM], gdt, kind="ExternalInput").ap()
    wp_d = nc.dram_tensor("proj_t", [DIM, DIM], gdt, kind="ExternalInput").ap()
    ident_d = nc.dram_tensor("ident", [HD, HD], gdt, kind="ExternalInput").ap()
    dg_d = [nc.dram_tensor(n, [NH, 9, HD, HD], cdt, kind="ExternalInput").ap()
            for n in ("diag1q", "diag1k", "diag2")]
    wv1_d = [nc.dram_tensor(n, [NH, HD, 9], f32, kind="ExternalInput").ap()
             for n in ("wvec1q", "wvec1k", "wvec2")]
    b1_d = [nc.dram_tensor(n, [DIM, 1], f32, kind="ExternalInput").ap()
            for n in ("sq_b", "sk_b", "dwc_b")]
    projb_d = nc.dram_tensor("proj_b", [DIM, 1], f32, kind="ExternalInput").ap()
    sew1_d = [nc.dram_tensor(n, [NH, HD, HD4], f32, kind="ExternalInput").ap()
              for n in ("se_w1q", "se_w1k")]
    seb1_d = [nc.dram_tensor(n, [NH, HD4, 1], f32, kind="ExternalInput").ap()
              for n in ("se_b1q", "se_b1k")]
    sew2_d = [nc.dram_tensor(n, [NH, HD4, HD], f32, kind="ExternalInput").ap()
              for n in ("se_w2q", "se_w2k")]
    seb2_d = [nc.dram_tensor(n, [NH, HD, 1], f32, kind="ExternalInput").ap()
              for n in ("se_b2q", "se_b2k")]

    with tile.TileContext(nc) as tc, ExitStack() as ctx:
        const = ctx.enter_context(tc.tile_pool(name="const", bufs=1))
        big = ctx.enter_context(tc.tile_pool(name="big", bufs=1))
        wpool = ctx.enter_context(tc.tile_pool(name="wpool", bufs=1))
        xpool = ctx.enter_context(tc.tile_pool(name="xpool", bufs=2))
        vpool = ctx.enter_context(tc.tile_pool(name="vpool", bufs=2))
        o2pool = ctx.enter_context(tc.tile_pool(name="o2pool", bufs=2))
        otpool = ctx.enter_context(tc.tile_pool(name="otpool", bufs=2))
        statpool = ctx.enter_context(tc.tile_pool(name="statpool", bufs=10))
        mmpool = ctx.enter_context(tc.tile_pool(name="mmpool", bufs=6, space="PSUM"))
        sepool = ctx.enter_context(tc.tile_pool(name="sepool", bufs=2, space="PSUM"))

        # ---------- persistent SBUF ----------
        # padded conv-domain buffers; 2-elem slop so padded-space tap reads
        # (offsets -WP-1 .. +WP+1) stay in bounds
        qpad = [big.tile([HD, PADN + 2], cdt, name=f"qpad{c}") for c in range(NH)]
        kpad = [big.tile([HD, PADN + 2], cdt, name=f"kpad{c}") for c in range(NH)]
        dwq = [big.tile([HD, PADN + 2], cdt, name=f"dwq{c}") for c in range(NH)]
        dwk = [big.tile([HD, PADN + 2], cdt, name=f"dwk{c}") for c in range(NH)]

        def pad3(t):
            return t[:, 1:1 + PADN].rearrange("p (h w) -> p h w", w=WP)

        qpad3, kpad3 = [pad3(t) for t in qpad], [pad3(t) for t in kpad]
        dwq3, dwk3 = [pad3(t) for t in dwq], [pad3(t) for t in dwk]

        # off-PE conv machinery: per tap, a full-tile scaled copy of the
        # source (TS on DVE runs 4x; ACT copy offloads it), then an
        # in-place TT add (2x, shift rides the second operand's slice --
        # measured insensitive to operand alignment).  STT is 1x always;
        # never use it on the big buffers.
        acc_g = big.tile([HD, PADN + 2], cdt, name="accg")
        tmpA = [big.tile([HD, PADN + 2], cdt, name=f"tmpA{i}") for i in range(3)]
        tmpS = big.tile([HD, PADN + 2], cdt, name="tmpS")

        # pad-zero invariant for the conv buffers: only the cells outside
        # the interior need zeroing (interiors are written before read),
        # so three small memsets per tile instead of a full-tile sweep.
        # The scratch tiles (acc_g/tmpA/tmpS) need no init at all.
        for tt in qpad + kpad + dwq + dwk:
            nc.vector.memset(tt[:, 0:1 + WP + 1], 0.0)
            nc.vector.memset(tt[:, 1 + 57 * WP:PADN + 2], 0.0)
            nc.vector.memset(
                tt[:, WP - 1:WP - 1 + 57 * WP].rearrange(
                    "p (h w) -> p h w", w=WP)[:, :, 0:3], 0.0)

        # small constants
        bias1 = [[const.tile([HD, 1], f32, name=f"b1_{ci}_{c}") for c in range(NH)]
                 for ci in range(3)]
        projb = [const.tile([HD, 1], f32, name=f"projb{c}") for c in range(NH)]

        def load_bias_consts():
            for c in range(NH):
                sl = slice(c * HD, (c + 1) * HD)
                for ci in range(3):
                    nc.sync.dma_start(bias1[ci][c], b1_d[ci][sl])
                nc.sync.dma_start(projb[c], projb_d[sl])
        wvec = [[const.tile([HD, 9], f32, name=f"wvec_{ci}_{c}") for c in range(NH)]
                for ci in range(3)]

        def load_wvec_consts():
            for ci in range(3):
                for c in range(NH):
                    nc.sync.dma_start(wvec[ci][c], wv1_d[ci][c])

        sew1 = [[const.tile([HD, HD4], f32, name=f"sew1_{br}_{c}") for c in range(NH)]
                for br in range(2)]
        seb1 = [[const.tile([HD4, 1], f32, name=f"seb1_{br}_{c}") for c in range(NH)]
                for br in range(2)]
        sew2 = [[const.tile([HD4, HD], f32, name=f"sew2_{br}_{c}") for c in range(NH)]
                for br in range(2)]
        seb2 = [[const.tile([HD, 1], f32, name=f"seb2_{br}_{c}") for c in range(NH)]
                for br in range(2)]

        def load_se_consts():
            for br in range(2):
                for c in range(NH):
                    nc.sync.dma_start(sew1[br][c], sew1_d[br][c])
                    nc.sync.dma_start(seb1[br][c], seb1_d[br][c])
                    nc.sync.dma_start(sew2[br][c], sew2_d[br][c])
                    nc.sync.dma_start(seb2[br][c], seb2_d[br][c])

        # persistent GEMM weights (loaded once, reused across samples;
        # DMA emission deferred into the schedule so the first sample's
        # x tiles aren't queued behind the full weight set)
        ident = const.tile([HD, HD], gdt, name="ident")

        def load_w(w_d, nm):
            w_sb = [[None] * NH for _ in range(NH)]
            for oc in range(NH):
                for kc in range(NH):
                    wt = wpool.tile([HD, HD], gdt, name=f"{nm}{kc}_{oc}")
                    nc.sync.dma_start(wt, w_d[kc * HD:(kc + 1) * HD,
                                              oc * HD:(oc + 1) * HD])
                    w_sb[kc][oc] = wt
            return w_sb

        wq_sb = wk_sb = wv_sb = wp_sb = None
        dg_sb = {}

        def load_dg():
            for ci in range(3):
                for oc in range(NH):
                    if any(a_of(b, ci, oc) == 'pe' for b in range(BL)):
                        dg_sb[(ci, oc)] = [
                            const.tile([HD, HD], cdt, name=f"dg{ci}_{oc}_{j}")
                            for j in range(9)]
                        for j in range(9):
                            nc.sync.dma_start(dg_sb[(ci, oc)][j],
                                              dg_d[ci][oc, j])

        def taps_flat_tile(tbuf, r0):
            """9 contiguous slices (full padded rows) for padded-space conv
            over output padded rows r0+1..r0+TH (tile granularity, PE)."""
            base = 1 + (r0 + 1) * WP
            return [tbuf[:, base + dy * WP + dx: base + dy * WP + dx + TPAD]
                    for (dy, dx) in TAPS]

        # accumulate window: elements [ALO, ALO+ACNT) -- 4B-aligned start,
        # superset of the interior; src windows [ALO+d, ALO+ACNT+d) stay
        # in bounds for every tap offset d in [-60, 60]
        ALO, ACNT = WP + 1, H * WP
        CPN = PADN + 2  # full scale-copy window

        def conv_head(ci, oc, srcb, dst3, b, stats, alpha):
            """One off-PE depthwise-conv head.  Per tap: full-tile scaled
            copy (first `alpha` non-base taps on ACT, rest as 4x TS on
            DVE), then 2x in-place TT add with the shift in the second
            operand's slice.  Final strided extraction on ACT adds the
            pooling accum.  Bias rides the base-tap TS."""
            assert alpha <= len(tmpA)
            bias = bias1[ci][oc]
            wv_ = wvec[ci][oc]
            deltas = [dy * WP + dx for (dy, dx) in TAPS]
            act_taps = list(range(1, 1 + alpha))
            ts_taps = list(range(1 + alpha, 9))
            # conv2 accumulates straight into dwq[oc] (dead after
            # phase15): interior is all phaseD reads, in-window pad cols
            # are re-zeroed before the next sample's phase15
            acc = dwq[oc] if ci == 2 else acc_g
            for i, j in enumerate(act_taps):
                nc.scalar.activation(tmpA[i % 3][:, 0:CPN],
                                     srcb[:, 0:CPN], AF.Copy,
                                     scale=wv_[:, j:j + 1])
            d0 = deltas[0]
            nc.vector.tensor_scalar(
                acc[:, ALO:ALO + ACNT],
                srcb[:, ALO + d0:ALO + d0 + ACNT],
                wv_[:, 0:1], bias, AL.mult, AL.add)
            for j in ts_taps:
                nc.vector.tensor_scalar(tmpS[:, 0:CPN], srcb[:, 0:CPN],
                                        wv_[:, j:j + 1], None, AL.mult)
                d = deltas[j]
                nc.vector.tensor_tensor(
                    acc[:, ALO:ALO + ACNT],
                    acc[:, ALO:ALO + ACNT],
                    tmpS[:, ALO + d:ALO + d + ACNT], AL.add)
            for i, j in enumerate(act_taps):
                d = deltas[j]
                nc.vector.tensor_tensor(
                    acc[:, ALO:ALO + ACNT],
                    acc[:, ALO:ALO + ACNT],
                    tmpA[i % 3][:, ALO + d:ALO + d + ACNT], AL.add)
            if ci == 2:
                return
            # strided interior extraction + pooling accum for conv1
            acc3 = pad3(acc_g)
            nc.scalar.activation(
                dst3[:, 1:1 + H, 1:1 + W],
                acc3[:, 1:1 + H, 1:1 + W],
                AF.Identity, bias=0.0, accum_out=stats[:, 0:1])

        def pe_conv_head(ci, oc, srcb, dst3, b, stats):
            """PE diag-matmul conv head (conv1 only: writes padded dst3
            with bias + accum pooling)."""
            dgs = dg_sb[(ci, oc)]
            for t in range(NT):
                r0 = t * TH
                ps = mmpool.tile([HD, TPAD], f32, tag="mm",
                                 name=f"c1{b}_{ci}_{t}_{oc}")
                for j, v in enumerate(taps_flat_tile(srcb, r0)):
                    nc.tensor.matmul(ps, dgs[j], v, start=(j == 0),
                                     stop=(j == 8))
                nc.scalar.activation(
                    dst3[:, 1 + r0:1 + r0 + TH, 1:1 + W],
                    ps.rearrange("p (h w) -> p h w", w=WP)[:, :, 1:1 + W],
                    AF.Identity, bias=bias1[ci][oc],
                    accum_out=stats[:, t:t + 1])

        def emit_se(b, br, oc, stats, pooled_w, s_scale):
            """SE chain for one head: pooled stats -> sigmoid scale."""
            pooled = const.tile([HD, 1], f32, tag="pooled", bufs=4,
                                name=f"pool{b}_{br}_{oc}")
            nc.vector.tensor_reduce(pooled, stats[:, 0:pooled_w],
                                    mybir.AxisListType.X, AL.add)
            ps1 = sepool.tile([HD4, 1], f32, tag="se", name=f"se1_{b}_{br}_{oc}")
            nc.tensor.matmul(ps1, sew1[br][oc], pooled, start=True, stop=True)
            hvec = const.tile([HD4, 1], f32, tag="hvec", bufs=4,
                              name=f"h{b}_{br}_{oc}")
            nc.scalar.activation(hvec, ps1, AF.Relu, bias=seb1[br][oc])
            ps2 = sepool.tile([HD, 1], f32, tag="se", name=f"se2_{b}_{br}_{oc}")
            nc.tensor.matmul(ps2, sew2[br][oc], hvec, start=True, stop=True)
            s_sb = const.tile([HD, 1], f32, tag="s_scale", bufs=16,
                              name=f"s{b}_{br}_{oc}")
            nc.scalar.activation(s_sb, ps2, AF.Sigmoid, bias=seb2[br][oc])
            s_scale[b][br][oc] = s_sb

        s_scale = [[[None] * NH for _ in range(2)] for _ in range(BL)]
        stats_t = {}

        def phaseA(b, br):
            """One branch's GEMM for sample b -> qpad/kpad."""
            if True:
                w_sb = wq_sb if br == 0 else wk_sb
                p3 = qpad3 if br == 0 else kpad3
                for t in range(NT):
                    r0 = t * TH
                    xt = []
                    for kc in range(NH):
                        xx = xpool.tile([HD, TN], gdt, tag=f"x{kc}",
                                        name=f"xa{kc}_b{b}_{br}_{t}")
                        nc.sync.dma_start(
                            xx.rearrange("p (h w) -> p h w", w=W),
                            x_d[b, kc * HD:(kc + 1) * HD, r0:r0 + TH, :])
                        xt.append(xx)
                    for oc in range(NH):
                        ps = mmpool.tile([HD, TN], f32, tag="mm",
                                         name=f"g{b}_{br}_{t}_{oc}")
                        for kc in range(NH):
                            nc.tensor.matmul(ps, w_sb[kc][oc], xt[kc],
                                             start=(kc == 0),
                                             stop=(kc == NH - 1))
                        nc.scalar.copy(
                            p3[oc][:, 1 + r0:1 + r0 + TH, 1:1 + W],
                            ps.rearrange("p (h w) -> p h w", w=W))

        def phaseB(b, br, which):
            """conv1 heads for one branch; `which` selects 'pe' or
            off-PE heads so PE conv work can be emitted ahead of the
            next sample's GEMM in the (in-order) PE queue.  SE chains
            are emitted later."""
            srcb = qpad if br == 0 else kpad
            dst3 = dwq3 if br == 0 else dwk3
            for oc in range(NH):
                eng = a_of(b, br, oc)
                if (eng == 'pe') != (which == 'pe'):
                    continue
                stats = statpool.tile([HD, NT], f32, tag="stats",
                                      name=f"st{b}_{br}_{oc}")
                if eng == 'pe':
                    pe_conv_head(br, oc, srcb[oc], dst3[oc], b, stats)
                else:
                    conv_head(br, oc, srcb[oc], dst3[oc], b, stats, eng)
                stats_t[(b, br, oc)] = stats

        def phaseB_se(b):
            for br in range(2):
                for oc in range(NH):
                    pw = NT if a_of(b, br, oc) == 'pe' else 1
                    emit_se(b, br, oc, stats_t[(b, br, oc)], pw, s_scale)

        def phase15(b):
            """m = s_q*dwq + s_k*dwk -> dwk, all TS/TT (STT is 1x)."""
            if b > 0:
                # previous sample's conv2 dirtied dwq's in-window pad
                # cols (w in {0,57,58}, rows 1..56); re-zero before the
                # full-tile TS reads them.  Pad rows 0/57 sit outside
                # the accumulate window and stayed zero.
                for oc in range(NH):
                    v = dwq[oc][:, WP - 1:WP - 1 + 57 * WP].rearrange(
                        "p (h w) -> p h w", w=WP)[:, :, 0:3]
                    nc.gpsimd.memset(v, 0.0)
            for oc in range(NH):
                nc.vector.tensor_scalar(acc_g[:, 0:CPN], dwk[oc][:, 0:CPN],
                                        s_scale[b][1][oc], None, AL.mult)
                nc.vector.tensor_scalar(dwk[oc][:, 0:CPN], dwq[oc][:, 0:CPN],
                                        s_scale[b][0][oc], None, AL.mult)
                nc.vector.tensor_tensor(dwk[oc][:, 0:CPN], dwk[oc][:, 0:CPN],
                                        acc_g[:, 0:CPN], AL.add)

        def phaseC_offpe(b):
            """Off-PE conv2 heads: m -> dwq (dwq is dead after phase15;
            its pads are permanently zero, so phaseD can read the conv2
            result from dwq3's interior)."""
            for oc in range(NH):
                eng = a_of(b, 2, oc)
                if eng == 'pe':
                    continue
                conv_head(2, oc, dwk[oc], dwq3[oc], b, None, eng)

        def phaseD(b):
            """Per row-tile: v GEMM, o2 = c2*v, proj GEMM + residual."""
            for t in range(NT):
                r0 = t * TH
                xt = []
                for kc in range(NH):
                    xx = xpool.tile([HD, TN], gdt, tag=f"x{kc}",
                                    name=f"xd{kc}_b{b}_{t}")
                    nc.sync.dma_start(
                        xx.rearrange("p (h w) -> p h w", w=W),
                        x_d[b, kc * HD:(kc + 1) * HD, r0:r0 + TH, :])
                    xt.append(xx)
                o2 = []
                for oc in range(NH):
                    ps = mmpool.tile([HD, TN], f32, tag="mm",
                                     name=f"v{b}_{t}_{oc}")
                    for kc in range(NH):
                        nc.tensor.matmul(ps, wv_sb[kc][oc], xt[kc],
                                         start=(kc == 0), stop=(kc == NH - 1))
                    vv = vpool.tile([HD, TN], gdt, tag="vt", bufs=6,
                                    name=f"vt{oc}_b{b}_{t}")
                    nc.scalar.copy(vv, ps)
                    if a_of(b, 2, oc) == 'pe':
                        # conv2 on PE for this head, this tile
                        ps2 = mmpool.tile([HD, TPAD], f32, tag="mm",
                                          name=f"c2{b}_{t}_{oc}")
                        for j, v in enumerate(taps_flat_tile(dwk[oc], r0)):
                            nc.tensor.matmul(ps2, dg_sb[(2, oc)][j], v,
                                             start=(j == 0), stop=(j == 8))
                        c2t = o2pool.tile([HD, TN], gdt, tag="c2t", bufs=4,
                                          name=f"c2t{oc}_b{b}_{t}")
                        nc.scalar.activation(
                            c2t.rearrange("p (h w) -> p h w", w=W),
                            ps2.rearrange("p (h w) -> p h w",
                                          w=WP)[:, :, 1:1 + W],
                            AF.Identity, bias=bias1[2][oc])
                        c2_sl = c2t.rearrange("p (h w) -> p h w", w=W)
                    else:
                        c2_sl = dwq3[oc][:, 1 + r0:1 + r0 + TH, 1:1 + W]
                    oo = o2pool.tile([HD, TN], gdt, tag="o2", bufs=6,
                                     name=f"o2_{oc}_b{b}_{t}")
                    nc.vector.tensor_mul(oo.rearrange("p (h w) -> p h w", w=W),
                                         c2_sl,
                                         vv.rearrange("p (h w) -> p h w", w=W))
                    o2.append(oo)
                for oc in range(NH):
                    ps = mmpool.tile([HD, TN], f32, tag="mm",
                                     name=f"p{b}_{t}_{oc}")
                    for kc in range(NH):
                        nc.tensor.matmul(ps, wp_sb[kc][oc], o2[kc],
                                         start=(kc == 0), stop=False)
                    # residual: accumulate x via identity matmul
                    nc.tensor.matmul(ps, ident, xt[oc], start=False,
                                     stop=True)
                    ot = otpool.tile([HD, TN], f32, tag="ot", bufs=2,
                                     name=f"ot{oc}_b{b}_{t}")
                    nc.scalar.activation(ot, ps, AF.Identity, bias=projb[oc])
                    nc.sync.dma_start(
                        out_d[b, oc * HD:(oc + 1) * HD, r0:r0 + TH, :],
                        ot.rearrange("p (h w) -> p h w", w=W))

        # ---- emission schedule: per branch, sample b's conv heads free
        # qpad/kpad, then sample b+1's GEMM for that branch refills it,
        # keeping the PE busy while the vector engines run the convs.
        # Weight DMAs are interleaved so the DMA queue serves the first
        # GEMM's x tiles early; bulk weights ride behind them ----
        wq_sb = load_w(wq_d, "wq")
        phaseA(0, 0)
        load_bias_consts()
        load_wvec_consts()
        wk_sb = load_w(wk_d, "wk")
        phaseA(0, 1)
        load_dg()
        wv_sb = load_w(wv_d, "wv")
        wp_sb = load_w(wp_d, "wp")
        nc.sync.dma_start(ident, ident_d)
        load_se_consts()
        for b in range(BL):
            phaseB(b, 0, 'off')   # conv1-q off-PE chains (DVE/ACT)
            phaseB(b, 0, 'pe')    # conv1-q PE heads
            phaseB(b, 1, 'pe')    # conv1-k PE heads: PE work that
                                  # covers the next q-GEMM's wait for
                                  # qpad to be freed by the q chains
            if b + 1 < BL:
                phaseA(b + 1, 0)  # next sample's q GEMM fills the PE
            phaseB(b, 1, 'off')   # conv1-k off-PE chains
            if b + 1 < BL:
                phaseA(b + 1, 1)
            phaseB_se(b)
            phase15(b)
            phaseC_offpe(b)
            phaseD(b)

    nc.compile()
    return nc


# ---------------------------------------------------------------------------
# host-side weight prep
# ---------------------------------------------------------------------------

def prep_weights(inputs, cfg):
    import ml_dtypes
    bf = ml_dtypes.bfloat16
    f32 = np.float32
    qkv_w = np.asarray(inputs['qkv_w'], f32)
    wq_t = np.ascontiguousarray(qkv_w[0:DIM].T).astype(bf)
    wk_t = np.ascontiguousarray(qkv_w[DIM:2 * DIM].T).astype(bf)
    wv_t = np.ascontiguousarray(qkv_w[2 * DIM:3 * DIM].T).astype(bf)
    proj_t = np.ascontiguousarray(np.asarray(inputs['proj_w'], f32).T).astype(bf)

    def diag_taps(wconv):
        w = np.asarray(wconv, f32).reshape(DIM, 9)
        out = np.zeros((NH, 9, HD, HD), f32)
        idx = np.arange(HD)
        for c in range(NH):
            for j in range(9):
                out[c, j, idx, idx] = w[c * HD:(c + 1) * HD, j]
        return out.astype(bf)

    def wvecs(wconv):
        w = np.asarray(wconv, f32).reshape(DIM, 9)
        return np.ascontiguousarray(w.reshape(NH, HD, 9))

    npix = cfg['H'] * W
    return dict(
        wq_t=wq_t, wk_t=wk_t, wv_t=wv_t, proj_t=proj_t,
        ident=np.eye(HD, dtype=f32).astype(bf),
        diag1q=diag_taps(inputs['sq_w']),
        diag1k=diag_taps(inputs['sk_w']),
        diag2=diag_taps(inputs['dwc_w']),
        wvec1q=wvecs(inputs['sq_w']),
        wvec1k=wvecs(inputs['sk_w']),
        wvec2=wvecs(inputs['dwc_w']),
        sq_b=np.asarray(inputs['sq_b'], f32).reshape(DIM, 1),
        sk_b=np.asarray(inputs['sk_b'], f32).reshape(DIM, 1),
        dwc_b=np.asarray(inputs['dwc_b'], f32).reshape(DIM, 1),
        proj_b=np.asarray(inputs['proj_b'], f32).reshape(DIM, 1),
        se_w1q=np.ascontiguousarray(
            np.asarray(inputs['cq_w1'], f32).transpose(0, 2, 1) / npix),
        se_b1q=np.asarray(inputs['cq_b1'], f32).reshape(NH, HD4, 1),
        se_w2q=np.ascontiguousarray(
            np.asarray(inputs['cq_w2'], f32).transpose(0, 2, 1)),
        se_b2q=np.asarray(inputs['cq_b2'], f32).reshape(NH, HD, 1),
        se_w1k=np.ascontiguousarray(
            np.asarray(inputs['ck_w1'], f32).transpose(0, 2, 1) / npix),
        se_b1k=np.asarray(inputs['ck_b1'], f32).reshape(NH, HD4, 1),
        se_w2k=np.ascontiguousarray(
            np.asarray(inputs['ck_w2'], f32).transpose(0, 2, 1)),
        se_b2k=np.asarray(inputs['ck_b2'], f32).reshape(NH, HD, 1),
    )


_CACHE = {}


def _get_compiled(cfg_key, cfg):
    if cfg_key not in _CACHE:
        _CACHE[cfg_key] = build_nc(cfg)
    return _CACHE[cfg_key]


def kernel(**inputs):
    import ml_dtypes
    from concourse import bass_utils
    cfg = default_cfg()
    nc = _get_compiled('main', cfg)
    w = prep_weights(inputs, cfg)
    x32 = np.asarray(inputs['x'], np.float32)
    x = x32.astype(ml_dtypes.bfloat16)
    BL = cfg['b_local']
    in_maps = []
    for core in range(N_CORES):
        m = dict(w)
        m['x'] = np.ascontiguousarray(x[core * BL:(core + 1) * BL])
        in_maps.append(m)
    res = bass_utils.run_bass_kernel_spmd(nc, in_maps, core_ids=list(range(N_CORES)))
    out = np.empty((B, DIM, H_FULL, W), np.float32)
    for core in range(N_CORES):
        out[core * BL:(core + 1) * BL] = res.results[core]['out']
    return out
